# revision 10
# baseline (speedup 1.0000x reference)
"""MoE FFN (grouped sigmoid top-k routing + shared expert) on 8 TRN2 NeuronCores.

Strategy: expert-parallel. Each core gets 2 of 16 routed experts plus 1/8 of
the shared expert (sharded along its hidden dim HS). x is replicated
(host-pre-transposed to [C, S] so every matmul contracts over the SBUF
partition dim). Routing is computed on-device, replicated on every core.
Each core emits a partial output [C, S]; the host sums the 8 partials and
transposes back.

dtypes: router matmuls run in full fp32 (top-k selection is sensitive to
input rounding); FFN matmuls run in fp32r (fp32 rounded to 11 mantissa bits,
full PE rate, ~1e-4 relative error).
"""

import numpy as np

import concourse.bacc as bacc
import concourse.mybir as mybir
from concourse import tile
from concourse.bass_utils import run_bass_kernel_spmd
from concourse.masks import make_identity

F32 = mybir.dt.float32
F32R = mybir.dt.float32r
AF = mybir.ActivationFunctionType
OP = mybir.AluOpType

# problem shapes (hardcoded; kernel.py must be self-contained)
B, T, C, H, HS = 2, 1024, 1024, 256, 2048
E, G, EPG = 16, 4, 4
TOPK = 4
NCORES = 8
S = B * T                  # 2048 tokens
EPC = E // NCORES          # 2 experts per core
HSL = HS // NCORES         # 256 shared-hidden rows per core
KC = C // 128              # 8 contraction chunks
NT = S // 128              # 16 token chunks
NSC = S // 512             # 4 moving (token) chunks of 512
NHC = H // 128             # 2 h chunks (same for HSL)
NCC = C // 128             # 8 output-row chunks


def _round_f32r(x: np.ndarray) -> np.ndarray:
    """Round fp32 to fp32r (RNE to 11 mantissa bits) — matches TRN2 PE."""
    u = np.ascontiguousarray(x, dtype=np.float32).view(np.uint32)
    u = u + 0x7FF + ((u >> 12) & 1)
    u = u & np.uint32(0xFFFFF000)
    return u.view(np.float32)


def build():
    nc = bacc.Bacc(
        "TRN2",
        target_bir_lowering=False,
        debug=False,
        enable_asserts=True,
        num_devices=NCORES,
    )
    # ---- DRAM I/O (per core) ----
    x_d = nc.declare_dram_parameter("xT", [C, S], F32, isOutput=False)
    rw_d = nc.declare_dram_parameter("rw", [128, 128], F32, isOutput=False)
    bias_d = nc.declare_dram_parameter("bias", [1, E], F32, isOutput=False)
    esel_d = nc.declare_dram_parameter("esel", [E, EPC * 128], F32R,
                                       isOutput=False)
    gw_d = nc.declare_dram_parameter("gw", [EPC, C, H], F32R, isOutput=False)
    uw_d = nc.declare_dram_parameter("uw", [EPC, C, H], F32R, isOutput=False)
    dw_d = nc.declare_dram_parameter("dw", [EPC, H, C], F32R, isOutput=False)
    sgw_d = nc.declare_dram_parameter("sgw", [C, HSL], F32R, isOutput=False)
    suw_d = nc.declare_dram_parameter("suw", [C, HSL], F32R, isOutput=False)
    sdw_d = nc.declare_dram_parameter("sdw", [HSL, C], F32R, isOutput=False)
    out_d = nc.declare_dram_parameter("out", [C, S], F32, isOutput=True)

    with tile.TileContext(nc) as tc:
        _emit(nc, tc, x_d, rw_d, bias_d, esel_d, gw_d, uw_d, dw_d,
              sgw_d, suw_d, sdw_d, out_d)
    nc.finalize()
    return nc


def _emit(nc, tc, x_d, rw_d, bias_d, esel_d, gw_d, uw_d, dw_d,
          sgw_d, suw_d, sdw_d, out_d):
    consts = tc.alloc_tile_pool(name="consts", bufs=1)
    ident = consts.tile([128, 128], F32)
    make_identity(nc, ident[:])
    rw = consts.tile([128, 128], F32)
    nc.sync.dma_start(rw[:], rw_d[:])
    bias_sb = consts.tile([1, E], F32)
    nc.sync.dma_start(bias_sb[:], bias_d[:])
    esel = consts.tile([E, EPC * 128], F32R)
    nc.sync.dma_start(esel[:], esel_d[:])
    # down-proj weights, resident (all 3 sources needed together in the
    # down pass): wd[src][p, hc*1024 + c] = dw[src][hc*128+p, c]
    wd = [consts.tile([128, NHC * C], F32R, tag=f"wd{i}", name=f"wd{i}")
          for i in range(3)]
    comb = consts.tile([128, NT * E], F32)       # combine weights [s, (t e)]

    # hw tiles [128, S] fp32r: (src, hc) -> silu(g)*u (* combine weight)
    hw_pool = tc.alloc_tile_pool(name="hw", bufs=1)
    hw = [[hw_pool.tile([128, S], F32R, tag=f"hw{src}{hc}",
                        name=f"hw{src}{hc}")
           for hc in range(NHC)] for src in range(3)]

    # x_r: fp32r copy of x, resident for all FFN matmuls
    xr_pool = tc.alloc_tile_pool(name="xr", bufs=1)
    x_r = xr_pool.tile([128, KC * S], F32R)

    # gate/up weight pool (opened early so expert 0's weights stream in
    # behind the first x chunk, during the router phase)
    wp = tc.alloc_tile_pool(name="wp", bufs=2)
    w_tiles = {}

    def load_w(src):
        wg = wp.tile([128, KC * H], F32R, tag="w", name=f"wg{src}")
        wu = wp.tile([128, KC * H], F32R, tag="w", name=f"wu{src}")
        gsrc = gw_d[src] if src < 2 else sgw_d
        usrc = uw_d[src] if src < 2 else suw_d
        nc.sync.dma_start(wg.rearrange("p (k h) -> p k h", k=KC),
                          gsrc.rearrange("(k p) h -> p k h", p=128))
        nc.sync.dma_start(wu.rearrange("p (k h) -> p k h", k=KC),
                          usrc.rearrange("(k p) h -> p k h", p=128))
        w_tiles[src] = (wg, wu)

    # ---------------- router + routing (scoped pools) ----------------
    with (
        tc.tile_pool(name="rt", bufs=1) as rt,
        tc.tile_pool(name="xs", bufs=2) as xs,
        tc.tile_pool(name="psl", bufs=NSC, space="PSUM") as psl,
        tc.tile_pool(name="pst", bufs=2, space="PSUM") as pst,
    ):
        scoresT = rt.tile([E, S], F32)
        pl = [psl.tile([E, 512], F32, tag="pl", name=f"pl{i}") for i in range(NSC)]
        for k in range(KC):
            xk = xs.tile([128, S], F32, tag="xk")
            nc.sync.dma_start(xk[:], x_d[k * 128:(k + 1) * 128, :])
            if k == KC - 2:
                load_w(0)
            # fp32r rounding copy for the FFN path
            nc.vector.tensor_copy(x_r[:, k * S:(k + 1) * S], xk[:])
            for sc in range(NSC):
                nc.tensor.matmul(
                    pl[sc][:],
                    rw[:, k * E:(k + 1) * E],
                    xk[:, sc * 512:(sc + 1) * 512],
                    start=(k == 0), stop=(k == KC - 1),
                )
        for sc in range(NSC):
            nc.scalar.activation(scoresT[:, sc * 512:(sc + 1) * 512], pl[sc][:],
                                 AF.Sigmoid)

        # transpose scores -> [s, (t e)] layout
        scores = rt.tile([128, NT * E], F32)
        for t in range(NT):
            pt = pst.tile([128, E], F32, tag="pt")
            nc.tensor.transpose(pt[:], scoresT[:, t * 128:(t + 1) * 128],
                                ident[:E, :E])
            nc.vector.tensor_copy(scores[:, t * E:(t + 1) * E], pt[:])

        # ---- routing math (all DVE), layout [128, (t=16, e=16)] ----
        sb = rt.tile([128, NT * E], F32)
        bias_exp = rt.tile([128, E], F32)
        nc.gpsimd.partition_broadcast(bias_exp[:], bias_sb[0:1, :])
        sbv = sb.rearrange("p (t e) -> p t e", t=NT)
        scv = scores.rearrange("p (t e) -> p t e", t=NT)
        nc.vector.tensor_add(
            sbv, scv, bias_exp[:, None, :].to_broadcast([128, NT, E]))

        # group top-2 sum over each group of 4: max over the 6 pairwise sums
        sbg = sb.rearrange("p (t g j) -> p t g j", t=NT, g=G)
        t2s = rt.tile([128, NT * G], F32)
        t2sv = t2s.rearrange("p (t g) -> p t g", t=NT)
        tmp = rt.tile([128, NT * G], F32)
        tmpv = tmp.rearrange("p (t g) -> p t g", t=NT)
        pairs = [(a, b) for a in range(EPG) for b in range(a + 1, EPG)]
        first = True
        for (a, b) in pairs:
            dst = t2sv if first else tmpv
            nc.vector.tensor_add(dst, sbg[:, :, :, a], sbg[:, :, :, b])
            if not first:
                nc.vector.tensor_max(t2sv, t2sv, tmpv)
            first = False

        # second-largest group score per token: max over pairwise mins
        m2 = rt.tile([128, NT], F32)
        m2t = rt.tile([128, NT], F32)
        gpairs = [(a, b) for a in range(G) for b in range(a + 1, G)]
        first = True
        for (a, b) in gpairs:
            dst = m2 if first else m2t
            nc.vector.tensor_tensor(dst[:], t2sv[:, :, a], t2sv[:, :, b], OP.min)
            if not first:
                nc.vector.tensor_max(m2[:], m2[:], m2t[:])
            first = False

        # penalty: -1e30 on experts whose group is not in the top 2
        pen = rt.tile([128, NT * G], F32)
        penv = pen.rearrange("p (t g) -> p t g", t=NT)
        nc.vector.tensor_tensor(
            penv, t2sv, m2[:, :, None].to_broadcast([128, NT, G]), OP.is_lt)
        nc.vector.tensor_scalar_mul(pen[:], pen[:], -1e30)

        sbm = rt.tile([128, NT * E], F32)
        sbmg = sbm.rearrange("p (t g j) -> p t g j", t=NT, g=G)
        nc.vector.tensor_add(
            sbmg, sbg, penv[:, :, :, None].to_broadcast([128, NT, G, EPG]))

        # 4th largest of the masked biased scores per token -> threshold
        m8 = rt.tile([128, NT * 8], F32)
        for t in range(NT):
            nc.vector.max(m8[:, t * 8:(t + 1) * 8], sbm[:, t * E:(t + 1) * E])
        v4 = m8.rearrange("p (t k) -> p t k", t=NT)[:, :, TOPK - 1]

        msk = rt.tile([128, NT * E], F32)
        mskv = msk.rearrange("p (t e) -> p t e", t=NT)
        sbmv = sbm.rearrange("p (t e) -> p t e", t=NT)
        nc.vector.tensor_tensor(
            mskv, sbmv, v4[:, :, None].to_broadcast([128, NT, E]), OP.is_ge)

        # weights: unbiased scores at selected positions, renormalized
        wm = rt.tile([128, NT * E], F32)
        nc.vector.tensor_mul(wm[:], scores[:], msk[:])
        ws = rt.tile([128, NT], F32)
        nc.vector.reduce_sum(ws[:], wm.rearrange("p (t e) -> p t e", t=NT),
                             axis=mybir.AxisListType.X)
        nc.vector.tensor_scalar_add(ws[:], ws[:], 1e-20)
        wr = rt.tile([128, NT], F32)
        nc.vector.reciprocal(wr[:], ws[:])
        combv = comb.rearrange("p (t e) -> p t e", t=NT)
        nc.vector.tensor_mul(
            combv, wm.rearrange("p (t e) -> p t e", t=NT),
            wr[:, :, None].to_broadcast([128, NT, E]))

    # ---------------- FFN ----------------
    # down-proj weight loads (needed only in the down pass; emitted here so
    # they don't delay the x/router DMAs)
    for src in range(2):
        nc.sync.dma_start(
            wd[src].rearrange("p (hc c) -> p hc c", hc=NHC),
            dw_d[src].rearrange("(hc p) c -> p hc c", p=128))
    nc.sync.dma_start(
        wd[2].rearrange("p (hc c) -> p hc c", hc=NHC),
        sdw_d.rearrange("(hc p) c -> p hc c", p=128))

    cp = tc.alloc_tile_pool(name="cp", bufs=1)
    with (
        tc.tile_pool(name="cb", bufs=1) as cbp,
        tc.tile_pool(name="hsb", bufs=2) as hsb,
        tc.tile_pool(name="psg", bufs=3, space="PSUM") as psg,
        tc.tile_pool(name="psu", bufs=3, space="PSUM") as psu,
    ):
        combT = None
        for src in range(3):
            if src not in w_tiles:
                load_w(src)
            wg, wu = w_tiles.pop(src)

            for hc in range(NHC):
                h_sb = hsb.tile([128, S], F32, tag="h")
                for sc in range(NSC):
                    pg = psg.tile([128, 512], F32, tag="pg")
                    pu = psu.tile([128, 512], F32, tag="pu")
                    for k in range(KC):
                        nc.tensor.matmul(
                            pg[:],
                            wg[:, k * H + hc * 128: k * H + (hc + 1) * 128],
                            x_r[:, k * S + sc * 512: k * S + (sc + 1) * 512],
                            start=(k == 0), stop=(k == KC - 1))
                    for k in range(KC):
                        nc.tensor.matmul(
                            pu[:],
                            wu[:, k * H + hc * 128: k * H + (hc + 1) * 128],
                            x_r[:, k * S + sc * 512: k * S + (sc + 1) * 512],
                            start=(k == 0), stop=(k == KC - 1))
                    sl = slice(sc * 512, (sc + 1) * 512)
                    nc.scalar.activation(h_sb[:, sl], pg[:], AF.Silu)
                    if src == 2:
                        # shared expert: no combine scaling; write f32r directly
                        nc.vector.tensor_mul(hw[src][hc][:, sl], h_sb[:, sl],
                                             pu[:])
                    else:
                        nc.vector.tensor_mul(h_sb[:, sl], h_sb[:, sl], pu[:])

                if src == 0 and combT is None:
                    # emit combine transposes after the first expert's g/u
                    # matmuls so the PE isn't stalled on the routing DVE chain
                    combT = cp.tile([E, S], F32R)
                    with tc.tile_pool(name="psct", bufs=2,
                                      space="PSUM") as psc:
                        for t in range(NT):
                            pct = psc.tile([E, 128], F32, tag="pct")
                            nc.tensor.transpose(
                                pct[:], comb[:, t * E:(t + 1) * E], ident[:])
                            nc.vector.tensor_copy(
                                combT[:, t * 128:(t + 1) * 128], pct[:])

                if src < 2 and hc == 0:
                    # broadcast this core's combine row across partitions by
                    # multiplying with a column-replicated one-hot (PE)
                    cb_exp = cbp.tile([128, S], F32, tag="cb", name="cb_exp")
                    with tc.tile_pool(name="pse2", bufs=2,
                                      space="PSUM") as pse2p:
                        for sc in range(NSC):
                            pe2 = pse2p.tile([128, 512], F32, tag="pe2")
                            nc.tensor.matmul(
                                pe2[:], esel[:, src * 128:(src + 1) * 128],
                                combT[:, sc * 512:(sc + 1) * 512],
                                start=True, stop=True)
                            nc.vector.tensor_copy(
                                cb_exp[:, sc * 512:(sc + 1) * 512], pe2[:])
                    cb_cur = cb_exp

                if src < 2:
                    nc.vector.tensor_mul(hw[src][hc][:], h_sb[:], cb_cur[:])

        if False:
            pass
    cp.release()
    wp.release()
    xr_pool.release()

    # ---------------- down projection ----------------
    with (
        tc.tile_pool(name="oso", bufs=2) as oso,
        tc.tile_pool(name="pso", bufs=4, space="PSUM") as pso,
    ):
        for cc in range(NCC):
            os_t = oso.tile([128, S], F32, tag="os")
            for sc in range(NSC):
                po = pso.tile([128, 512], F32, tag="po")
                idx = 0
                for src in range(3):
                    for hc in range(NHC):
                        nc.tensor.matmul(
                            po[:],
                            wd[src][:, hc * C + cc * 128: hc * C + (cc + 1) * 128],
                            hw[src][hc][:, sc * 512:(sc + 1) * 512],
                            start=(idx == 0), stop=(idx == 5))
                        idx += 1
                nc.vector.tensor_copy(os_t[:, sc * 512:(sc + 1) * 512], po[:])
            nc.sync.dma_start(out_d[cc * 128:(cc + 1) * 128, :], os_t[:])

    hw_pool.release()
    consts.release()


_NC_CACHE = {}


def _get_nc():
    if "nc" not in _NC_CACHE:
        _NC_CACHE["nc"] = build()
    return _NC_CACHE["nc"]


def make_in_maps(x, router_w, correction_bias, gate_w, up_w, down_w,
                 shared_gate_w, shared_up_w, shared_down_w):
    x = np.asarray(x, dtype=np.float32)
    xT = np.ascontiguousarray(x.reshape(S, C).T)                 # [C, S]
    rwT = np.asarray(router_w, dtype=np.float32).T               # [C, E]
    rw_pk = np.ascontiguousarray(
        rwT.reshape(KC, 128, E).transpose(1, 0, 2).reshape(128, KC * E))
    bias = np.asarray(correction_bias, dtype=np.float32).reshape(1, E)
    sgT = np.asarray(shared_gate_w, dtype=np.float32).T          # [C, HS]
    suT = np.asarray(shared_up_w, dtype=np.float32).T            # [C, HS]
    sdT = np.asarray(shared_down_w, dtype=np.float32).T          # [HS, C]
    gate_w = np.asarray(gate_w, dtype=np.float32)
    up_w = np.asarray(up_w, dtype=np.float32)
    down_w = np.asarray(down_w, dtype=np.float32)

    in_maps = []
    for c in range(NCORES):
        es = slice(c * EPC, (c + 1) * EPC)
        hs = slice(c * HSL, (c + 1) * HSL)
        esel = np.zeros((E, EPC * 128), np.float32)
        esel[c * EPC, 0:128] = 1.0
        esel[c * EPC + 1, 128:256] = 1.0
        in_maps.append({
            "xT": xT,
            "rw": rw_pk,
            "bias": bias,
            "esel": esel,
            "gw": _round_f32r(gate_w[es]),
            "uw": _round_f32r(up_w[es]),
            "dw": _round_f32r(down_w[es]),
            "sgw": _round_f32r(sgT[:, hs]),
            "suw": _round_f32r(suT[:, hs]),
            "sdw": _round_f32r(sdT[hs, :]),
        })
    return in_maps


def kernel(x, router_w, correction_bias, gate_w, up_w, down_w,
           shared_gate_w, shared_up_w, shared_down_w):
    in_maps = make_in_maps(x, router_w, correction_bias, gate_w, up_w, down_w,
                           shared_gate_w, shared_up_w, shared_down_w)
    nc = _get_nc()
    res = run_bass_kernel_spmd(nc, in_maps, list(range(NCORES)))
    acc = np.zeros((C, S), np.float64)
    for c in range(NCORES):
        acc += res.results[c]["out"].astype(np.float64)
    return np.ascontiguousarray(acc.T).astype(np.float32).reshape(B, T, C)


# revision 13
# speedup vs baseline: 1.0756x; 1.0756x over previous
"""MoE FFN (grouped sigmoid top-k routing + shared expert) on 8 TRN2 NeuronCores.

Strategy: expert-parallel. Each core gets 2 of 16 routed experts plus 1/8 of
the shared expert (sharded along its hidden dim HS). x is replicated
(host-pre-transposed to [C, S] so every matmul contracts over the SBUF
partition dim). Routing is computed on-device, replicated on every core.
Each core emits a partial output [C, S]; the host sums the 8 partials and
transposes back.

dtypes: router matmuls run in full fp32 (top-k selection is sensitive to
input rounding); FFN matmuls run in fp32r (fp32 rounded to 11 mantissa bits,
full PE rate, ~1e-4 relative error).
"""

import numpy as np

import concourse.bacc as bacc
import concourse.mybir as mybir
from concourse import tile
from concourse.bass_utils import run_bass_kernel_spmd
from concourse.masks import make_identity

F32 = mybir.dt.float32
F32R = mybir.dt.float32r
AF = mybir.ActivationFunctionType
OP = mybir.AluOpType

# problem shapes (hardcoded; kernel.py must be self-contained)
B, T, C, H, HS = 2, 1024, 1024, 256, 2048
E, G, EPG = 16, 4, 4
TOPK = 4
NCORES = 8
S = B * T                  # 2048 tokens
EPC = E // NCORES          # 2 experts per core
HSL = HS // NCORES         # 256 shared-hidden rows per core
KC = C // 128              # 8 contraction chunks
NT = S // 128              # 16 token chunks
NSC = S // 512             # 4 moving (token) chunks of 512
NHC = H // 128             # 2 h chunks (same for HSL)
NCC = C // 128             # 8 output-row chunks


def _round_f32r(x: np.ndarray) -> np.ndarray:
    """Round fp32 to fp32r (RNE to 11 mantissa bits) — matches TRN2 PE."""
    u = np.ascontiguousarray(x, dtype=np.float32).view(np.uint32)
    u = u + 0x7FF + ((u >> 12) & 1)
    u = u & np.uint32(0xFFFFF000)
    return u.view(np.float32)


def build():
    nc = bacc.Bacc(
        "TRN2",
        target_bir_lowering=False,
        debug=False,
        enable_asserts=True,
        num_devices=NCORES,
    )
    # ---- DRAM I/O (per core) ----
    x_d = nc.declare_dram_parameter("xT", [C, S], F32, isOutput=False)
    rw_d = nc.declare_dram_parameter("rw", [128, 128], F32, isOutput=False)
    bias_d = nc.declare_dram_parameter("bias", [1, E], F32, isOutput=False)
    esel_d = nc.declare_dram_parameter("esel", [E, EPC * 128], F32R,
                                       isOutput=False)
    gw_d = nc.declare_dram_parameter("gw", [EPC, C, H], F32R, isOutput=False)
    uw_d = nc.declare_dram_parameter("uw", [EPC, C, H], F32R, isOutput=False)
    dw_d = nc.declare_dram_parameter("dw", [EPC, H, C], F32R, isOutput=False)
    sgw_d = nc.declare_dram_parameter("sgw", [C, HSL], F32R, isOutput=False)
    suw_d = nc.declare_dram_parameter("suw", [C, HSL], F32R, isOutput=False)
    sdw_d = nc.declare_dram_parameter("sdw", [HSL, C], F32R, isOutput=False)
    out_d = nc.declare_dram_parameter("out", [C, S], F32, isOutput=True)

    with tile.TileContext(nc) as tc:
        _emit(nc, tc, x_d, rw_d, bias_d, esel_d, gw_d, uw_d, dw_d,
              sgw_d, suw_d, sdw_d, out_d)
    nc.finalize()
    return nc


def _emit(nc, tc, x_d, rw_d, bias_d, esel_d, gw_d, uw_d, dw_d,
          sgw_d, suw_d, sdw_d, out_d):
    consts = tc.alloc_tile_pool(name="consts", bufs=1)
    ident = consts.tile([128, 128], F32)
    make_identity(nc, ident[:])
    rw = consts.tile([128, 128], F32)
    nc.sync.dma_start(rw[:], rw_d[:])
    bias_sb = consts.tile([1, E], F32)
    nc.sync.dma_start(bias_sb[:], bias_d[:])
    esel = consts.tile([E, EPC * 128], F32R)
    nc.sync.dma_start(esel[:], esel_d[:])
    # down-proj weights, resident (all 3 sources needed together in the
    # down pass): wd[src][p, hc*1024 + c] = dw[src][hc*128+p, c]
    wd = [consts.tile([128, NHC * C], F32R, tag=f"wd{i}", name=f"wd{i}")
          for i in range(3)]
    comb = consts.tile([128, NT * E], F32)       # combine weights [s, (t e)]

    # hw tiles [128, S] fp32r: (src, hc) -> silu(g)*u (* combine weight)
    hw_pool = tc.alloc_tile_pool(name="hw", bufs=1)
    hw = [[hw_pool.tile([128, S], F32R, tag=f"hw{src}{hc}",
                        name=f"hw{src}{hc}")
           for hc in range(NHC)] for src in range(3)]

    # x_r: fp32r copy of x, resident for all FFN matmuls
    xr_pool = tc.alloc_tile_pool(name="xr", bufs=1)
    x_r = xr_pool.tile([128, KC * S], F32R)

    # gate/up weight pool (opened early so expert 0's weights stream in
    # behind the first x chunk, during the router phase)
    wp = tc.alloc_tile_pool(name="wp", bufs=2)
    w_tiles = {}

    def load_w(src):
        # one [128, KC*128] tile per (proj, hc): finer slot rotation lets the
        # next source's first-half weights stream while the current source is
        # still computing its second half
        tiles = {}
        for proj, wsrc in (("g", gw_d[src] if src < 2 else sgw_d),
                           ("u", uw_d[src] if src < 2 else suw_d)):
            for hc in range(NHC):
                wt = wp.tile([128, KC * 128], F32R, tag=f"{proj}{hc}",
                             name=f"w{proj}{src}{hc}")
                nc.sync.dma_start(
                    wt.rearrange("p (k h) -> p k h", k=KC),
                    wsrc.rearrange("(k p) h -> p k h", p=128)[
                        :, :, hc * 128:(hc + 1) * 128])
                tiles[(proj, hc)] = wt
        w_tiles[src] = tiles

    # ---------------- router + routing (scoped pools) ----------------
    with (
        tc.tile_pool(name="rt", bufs=1) as rt,
        tc.tile_pool(name="xs", bufs=2) as xs,
        tc.tile_pool(name="psl", bufs=NSC, space="PSUM") as psl,
        tc.tile_pool(name="pst", bufs=2, space="PSUM") as pst,
    ):
        scoresT = rt.tile([E, S], F32)
        pl = [psl.tile([E, 512], F32, tag="pl", name=f"pl{i}") for i in range(NSC)]
        for k in range(KC):
            xk = xs.tile([128, S], F32, tag="xk")
            nc.sync.dma_start(xk[:, :S // 2],
                              x_d[k * 128:(k + 1) * 128, :S // 2])
            nc.sync.dma_start(xk[:, S // 2:],
                              x_d[k * 128:(k + 1) * 128, S // 2:])
            # fp32r rounding copy for the FFN path
            nc.vector.tensor_copy(x_r[:, k * S:(k + 1) * S], xk[:])
            for sc in range(NSC):
                nc.tensor.matmul(
                    pl[sc][:],
                    rw[:, k * E:(k + 1) * E],
                    xk[:, sc * 512:(sc + 1) * 512],
                    start=(k == 0), stop=(k == KC - 1),
                )
        load_w(0)
        for sc in range(NSC):
            nc.scalar.activation(scoresT[:, sc * 512:(sc + 1) * 512], pl[sc][:],
                                 AF.Sigmoid)

        # transpose scores -> [s, (t e)] layout
        scores = rt.tile([128, NT * E], F32)
        for t in range(NT):
            pt = pst.tile([128, E], F32, tag="pt")
            nc.tensor.transpose(pt[:], scoresT[:, t * 128:(t + 1) * 128],
                                ident[:E, :E])
            nc.vector.tensor_copy(scores[:, t * E:(t + 1) * E], pt[:])

        # ---- routing math (all DVE), layout [128, (t=16, e=16)] ----
        sb = rt.tile([128, NT * E], F32)
        bias_exp = rt.tile([128, E], F32)
        nc.gpsimd.partition_broadcast(bias_exp[:], bias_sb[0:1, :])
        sbv = sb.rearrange("p (t e) -> p t e", t=NT)
        scv = scores.rearrange("p (t e) -> p t e", t=NT)
        nc.vector.tensor_add(
            sbv, scv, bias_exp[:, None, :].to_broadcast([128, NT, E]))

        # group top-2 sum over each group of 4: max over the 6 pairwise sums
        sbg = sb.rearrange("p (t g j) -> p t g j", t=NT, g=G)
        t2s = rt.tile([128, NT * G], F32)
        t2sv = t2s.rearrange("p (t g) -> p t g", t=NT)
        tmp = rt.tile([128, NT * G], F32)
        tmpv = tmp.rearrange("p (t g) -> p t g", t=NT)
        pairs = [(a, b) for a in range(EPG) for b in range(a + 1, EPG)]
        first = True
        for (a, b) in pairs:
            dst = t2sv if first else tmpv
            nc.vector.tensor_add(dst, sbg[:, :, :, a], sbg[:, :, :, b])
            if not first:
                nc.vector.tensor_max(t2sv, t2sv, tmpv)
            first = False

        # second-largest group score per token: max over pairwise mins
        m2 = rt.tile([128, NT], F32)
        m2t = rt.tile([128, NT], F32)
        gpairs = [(a, b) for a in range(G) for b in range(a + 1, G)]
        first = True
        for (a, b) in gpairs:
            dst = m2 if first else m2t
            nc.vector.tensor_tensor(dst[:], t2sv[:, :, a], t2sv[:, :, b], OP.min)
            if not first:
                nc.vector.tensor_max(m2[:], m2[:], m2t[:])
            first = False

        # penalty: -1e30 on experts whose group is not in the top 2
        pen = rt.tile([128, NT * G], F32)
        penv = pen.rearrange("p (t g) -> p t g", t=NT)
        nc.vector.tensor_tensor(
            penv, t2sv, m2[:, :, None].to_broadcast([128, NT, G]), OP.is_lt)
        nc.vector.tensor_scalar_mul(pen[:], pen[:], -1e30)

        sbm = rt.tile([128, NT * E], F32)
        sbmg = sbm.rearrange("p (t g j) -> p t g j", t=NT, g=G)
        nc.vector.tensor_add(
            sbmg, sbg, penv[:, :, :, None].to_broadcast([128, NT, G, EPG]))

        # 4th largest of the masked biased scores per token -> threshold
        m8 = rt.tile([128, NT * 8], F32)
        for t in range(NT):
            nc.vector.max(m8[:, t * 8:(t + 1) * 8], sbm[:, t * E:(t + 1) * E])
        v4 = m8.rearrange("p (t k) -> p t k", t=NT)[:, :, TOPK - 1]

        msk = rt.tile([128, NT * E], F32)
        mskv = msk.rearrange("p (t e) -> p t e", t=NT)
        sbmv = sbm.rearrange("p (t e) -> p t e", t=NT)
        nc.vector.tensor_tensor(
            mskv, sbmv, v4[:, :, None].to_broadcast([128, NT, E]), OP.is_ge)

        # weights: unbiased scores at selected positions, renormalized
        wm = rt.tile([128, NT * E], F32)
        nc.vector.tensor_mul(wm[:], scores[:], msk[:])
        ws = rt.tile([128, NT], F32)
        nc.vector.reduce_sum(ws[:], wm.rearrange("p (t e) -> p t e", t=NT),
                             axis=mybir.AxisListType.X)
        nc.vector.tensor_scalar_add(ws[:], ws[:], 1e-20)
        wr = rt.tile([128, NT], F32)
        nc.vector.reciprocal(wr[:], ws[:])
        combv = comb.rearrange("p (t e) -> p t e", t=NT)
        nc.vector.tensor_mul(
            combv, wm.rearrange("p (t e) -> p t e", t=NT),
            wr[:, :, None].to_broadcast([128, NT, E]))

    # ---------------- FFN ----------------
    # down-proj weight loads (needed only in the down pass; emitted here so
    # they don't delay the x/router DMAs)
    for src in range(2):
        nc.sync.dma_start(
            wd[src].rearrange("p (hc c) -> p hc c", hc=NHC),
            dw_d[src].rearrange("(hc p) c -> p hc c", p=128))
    nc.sync.dma_start(
        wd[2].rearrange("p (hc c) -> p hc c", hc=NHC),
        sdw_d.rearrange("(hc p) c -> p hc c", p=128))

    cp = tc.alloc_tile_pool(name="cp", bufs=1)
    with (
        tc.tile_pool(name="cb", bufs=1) as cbp,
        tc.tile_pool(name="hsb", bufs=2) as hsb,
        tc.tile_pool(name="psg", bufs=3, space="PSUM") as psg,
        tc.tile_pool(name="psu", bufs=3, space="PSUM") as psu,
    ):
        combT = None
        for src in range(3):
            if src not in w_tiles:
                load_w(src)
            wt = w_tiles.pop(src)

            for hc in range(NHC):
                h_sb = hsb.tile([128, S], F32, tag="h")
                for sc in range(NSC):
                    pg = psg.tile([128, 512], F32, tag="pg")
                    pu = psu.tile([128, 512], F32, tag="pu")
                    for k in range(KC):
                        nc.tensor.matmul(
                            pg[:],
                            wt[("g", hc)][:, k * 128:(k + 1) * 128],
                            x_r[:, k * S + sc * 512: k * S + (sc + 1) * 512],
                            start=(k == 0), stop=(k == KC - 1))
                    for k in range(KC):
                        nc.tensor.matmul(
                            pu[:],
                            wt[("u", hc)][:, k * 128:(k + 1) * 128],
                            x_r[:, k * S + sc * 512: k * S + (sc + 1) * 512],
                            start=(k == 0), stop=(k == KC - 1))
                    sl = slice(sc * 512, (sc + 1) * 512)
                    nc.scalar.activation(h_sb[:, sl], pg[:], AF.Silu)
                    if src == 2:
                        # shared expert: no combine scaling; write f32r directly
                        nc.vector.tensor_mul(hw[src][hc][:, sl], h_sb[:, sl],
                                             pu[:])
                    else:
                        nc.vector.tensor_mul(h_sb[:, sl], h_sb[:, sl], pu[:])

                if src == 0 and combT is None:
                    # emit combine transposes after the first expert's g/u
                    # matmuls so the PE isn't stalled on the routing DVE chain
                    combT = cp.tile([E, S], F32R)
                    with tc.tile_pool(name="psct", bufs=2,
                                      space="PSUM") as psc:
                        for t in range(NT):
                            pct = psc.tile([E, 128], F32, tag="pct")
                            nc.tensor.transpose(
                                pct[:], comb[:, t * E:(t + 1) * E], ident[:])
                            nc.vector.tensor_copy(
                                combT[:, t * 128:(t + 1) * 128], pct[:])

                if src < 2 and hc == 0:
                    # broadcast this core's combine row across partitions by
                    # multiplying with a column-replicated one-hot (PE)
                    cb_exp = cbp.tile([128, S], F32, tag="cb", name="cb_exp")
                    with tc.tile_pool(name="pse2", bufs=2,
                                      space="PSUM") as pse2p:
                        for sc in range(NSC):
                            pe2 = pse2p.tile([128, 512], F32, tag="pe2")
                            nc.tensor.matmul(
                                pe2[:], esel[:, src * 128:(src + 1) * 128],
                                combT[:, sc * 512:(sc + 1) * 512],
                                start=True, stop=True)
                            nc.vector.tensor_copy(
                                cb_exp[:, sc * 512:(sc + 1) * 512], pe2[:])
                    cb_cur = cb_exp

                if src < 2:
                    nc.vector.tensor_mul(hw[src][hc][:], h_sb[:], cb_cur[:])

    cp.release()
    wp.release()
    xr_pool.release()

    # ---------------- down projection ----------------
    with (
        tc.tile_pool(name="oso", bufs=2) as oso,
        tc.tile_pool(name="pso", bufs=4, space="PSUM") as pso,
    ):
        for cc in range(NCC):
            os_t = oso.tile([128, S], F32, tag="os")
            for sc in range(NSC):
                po = pso.tile([128, 512], F32, tag="po")
                idx = 0
                for src in range(3):
                    for hc in range(NHC):
                        nc.tensor.matmul(
                            po[:],
                            wd[src][:, hc * C + cc * 128: hc * C + (cc + 1) * 128],
                            hw[src][hc][:, sc * 512:(sc + 1) * 512],
                            start=(idx == 0), stop=(idx == 5))
                        idx += 1
                nc.vector.tensor_copy(os_t[:, sc * 512:(sc + 1) * 512], po[:])
                if cc == NCC - 1:
                    nc.sync.dma_start(
                        out_d[cc * 128:(cc + 1) * 128,
                              sc * 512:(sc + 1) * 512],
                        os_t[:, sc * 512:(sc + 1) * 512])
            if cc < NCC - 1:
                nc.sync.dma_start(out_d[cc * 128:(cc + 1) * 128, :], os_t[:])

    hw_pool.release()
    consts.release()


_NC_CACHE = {}


def _get_nc():
    if "nc" not in _NC_CACHE:
        _NC_CACHE["nc"] = build()
    return _NC_CACHE["nc"]


def make_in_maps(x, router_w, correction_bias, gate_w, up_w, down_w,
                 shared_gate_w, shared_up_w, shared_down_w):
    x = np.asarray(x, dtype=np.float32)
    xT = np.ascontiguousarray(x.reshape(S, C).T)                 # [C, S]
    rwT = np.asarray(router_w, dtype=np.float32).T               # [C, E]
    rw_pk = np.ascontiguousarray(
        rwT.reshape(KC, 128, E).transpose(1, 0, 2).reshape(128, KC * E))
    bias = np.asarray(correction_bias, dtype=np.float32).reshape(1, E)
    sgT = np.asarray(shared_gate_w, dtype=np.float32).T          # [C, HS]
    suT = np.asarray(shared_up_w, dtype=np.float32).T            # [C, HS]
    sdT = np.asarray(shared_down_w, dtype=np.float32).T          # [HS, C]
    gate_w = np.asarray(gate_w, dtype=np.float32)
    up_w = np.asarray(up_w, dtype=np.float32)
    down_w = np.asarray(down_w, dtype=np.float32)

    in_maps = []
    for c in range(NCORES):
        es = slice(c * EPC, (c + 1) * EPC)
        hs = slice(c * HSL, (c + 1) * HSL)
        esel = np.zeros((E, EPC * 128), np.float32)
        esel[c * EPC, 0:128] = 1.0
        esel[c * EPC + 1, 128:256] = 1.0
        in_maps.append({
            "xT": xT,
            "rw": rw_pk,
            "bias": bias,
            "esel": esel,
            "gw": _round_f32r(gate_w[es]),
            "uw": _round_f32r(up_w[es]),
            "dw": _round_f32r(down_w[es]),
            "sgw": _round_f32r(sgT[:, hs]),
            "suw": _round_f32r(suT[:, hs]),
            "sdw": _round_f32r(sdT[hs, :]),
        })
    return in_maps


def kernel(x, router_w, correction_bias, gate_w, up_w, down_w,
           shared_gate_w, shared_up_w, shared_down_w):
    in_maps = make_in_maps(x, router_w, correction_bias, gate_w, up_w, down_w,
                           shared_gate_w, shared_up_w, shared_down_w)
    nc = _get_nc()
    res = run_bass_kernel_spmd(nc, in_maps, list(range(NCORES)))
    acc = np.zeros((C, S), np.float64)
    for c in range(NCORES):
        acc += res.results[c]["out"].astype(np.float64)
    return np.ascontiguousarray(acc.T).astype(np.float32).reshape(B, T, C)


# revision 16
# speedup vs baseline: 1.0908x; 1.0141x over previous
"""MoE FFN (grouped sigmoid top-k routing + shared expert) on 8 TRN2 NeuronCores.

Strategy: expert-parallel. Each core gets 2 of 16 routed experts plus 1/8 of
the shared expert (sharded along its hidden dim HS). x is replicated
(host-pre-transposed to [C, S] so every matmul contracts over the SBUF
partition dim). Routing is computed on-device, replicated on every core.
Each core emits a partial output [C, S]; the host sums the 8 partials and
transposes back.

dtypes: router matmuls run in full fp32 (top-k selection is sensitive to
input rounding); FFN matmuls run in fp32r (fp32 rounded to 11 mantissa bits,
full PE rate, ~1e-4 relative error).
"""

import numpy as np

import concourse.bacc as bacc
import concourse.mybir as mybir
from concourse import tile
from concourse.bass_utils import run_bass_kernel_spmd
from concourse.masks import make_identity

F32 = mybir.dt.float32
F32R = mybir.dt.float32r
AF = mybir.ActivationFunctionType
OP = mybir.AluOpType

# problem shapes (hardcoded; kernel.py must be self-contained)
B, T, C, H, HS = 2, 1024, 1024, 256, 2048
E, G, EPG = 16, 4, 4
TOPK = 4
NCORES = 8
S = B * T                  # 2048 tokens
EPC = E // NCORES          # 2 experts per core
HSL = HS // NCORES         # 256 shared-hidden rows per core
KC = C // 128              # 8 contraction chunks
NT = S // 128              # 16 token chunks
NSC = S // 512             # 4 moving (token) chunks of 512
NHC = H // 128             # 2 h chunks (same for HSL)
NCC = C // 128             # 8 output-row chunks


def _round_f32r(x: np.ndarray) -> np.ndarray:
    """Round fp32 to fp32r (RNE to 11 mantissa bits) — matches TRN2 PE."""
    u = np.ascontiguousarray(x, dtype=np.float32).view(np.uint32)
    u = u + 0x7FF + ((u >> 12) & 1)
    u = u & np.uint32(0xFFFFF000)
    return u.view(np.float32)


def build():
    nc = bacc.Bacc(
        "TRN2",
        target_bir_lowering=False,
        debug=False,
        enable_asserts=True,
        num_devices=NCORES,
    )
    # ---- DRAM I/O (per core) ----
    x_d = nc.declare_dram_parameter("xT", [C, S], F32, isOutput=False)
    rw_d = nc.declare_dram_parameter("rw", [128, 128], F32, isOutput=False)
    bias_d = nc.declare_dram_parameter("bias", [1, E], F32, isOutput=False)
    esel_d = nc.declare_dram_parameter("esel", [E, EPC * 128], F32R,
                                       isOutput=False)
    gw_d = nc.declare_dram_parameter("gw", [EPC, C, H], F32R, isOutput=False)
    uw_d = nc.declare_dram_parameter("uw", [EPC, C, H], F32R, isOutput=False)
    dw_d = nc.declare_dram_parameter("dw", [EPC, H, C], F32R, isOutput=False)
    sgw_d = nc.declare_dram_parameter("sgw", [C, HSL], F32R, isOutput=False)
    suw_d = nc.declare_dram_parameter("suw", [C, HSL], F32R, isOutput=False)
    sdw_d = nc.declare_dram_parameter("sdw", [HSL, C], F32R, isOutput=False)
    out_d = nc.declare_dram_parameter("out", [C, S], F32, isOutput=True)

    with tile.TileContext(nc) as tc:
        _emit(nc, tc, x_d, rw_d, bias_d, esel_d, gw_d, uw_d, dw_d,
              sgw_d, suw_d, sdw_d, out_d)
    nc.finalize()
    return nc


def _emit(nc, tc, x_d, rw_d, bias_d, esel_d, gw_d, uw_d, dw_d,
          sgw_d, suw_d, sdw_d, out_d):
    consts = tc.alloc_tile_pool(name="consts", bufs=1)
    ident = consts.tile([128, 128], F32)
    make_identity(nc, ident[:])
    rw = consts.tile([128, 128], F32)
    nc.sync.dma_start(rw[:], rw_d[:])
    bias_sb = consts.tile([1, E], F32)
    nc.sync.dma_start(bias_sb[:], bias_d[:])
    esel = consts.tile([E, EPC * 128], F32R)
    nc.sync.dma_start(esel[:], esel_d[:])
    # down-proj weights, resident (all 3 sources needed together in the
    # down pass): wd[src][p, hc*1024 + c] = dw[src][hc*128+p, c]
    wd = [consts.tile([128, NHC * C], F32R, tag=f"wd{i}", name=f"wd{i}")
          for i in range(3)]
    comb = consts.tile([128, NT * E], F32)       # combine weights [s, (t e)]

    # hw tiles [128, S] fp32r: (src, hc) -> silu(g)*u (* combine weight)
    hw_pool = tc.alloc_tile_pool(name="hw", bufs=1)
    hw = [[hw_pool.tile([128, S], F32R, tag=f"hw{src}{hc}",
                        name=f"hw{src}{hc}")
           for hc in range(NHC)] for src in range(3)]

    # x_r: fp32r copy of x, resident for all FFN matmuls
    xr_pool = tc.alloc_tile_pool(name="xr", bufs=1)
    x_r = xr_pool.tile([128, KC * S], F32R)

    # gate/up weight pool (opened early so expert 0's weights stream in
    # behind the first x chunk, during the router phase)
    wp = tc.alloc_tile_pool(name="wp", bufs=2)
    w_tiles = {}

    def load_w(src):
        # one [128, KC*128] tile per (proj, hc): finer slot rotation lets the
        # next source's first-half weights stream while the current source is
        # still computing its second half
        tiles = {}
        for proj, wsrc in (("g", gw_d[src] if src < 2 else sgw_d),
                           ("u", uw_d[src] if src < 2 else suw_d)):
            for hc in range(NHC):
                wt = wp.tile([128, KC * 128], F32R, tag=f"{proj}{hc}",
                             name=f"w{proj}{src}{hc}")
                nc.sync.dma_start(
                    wt.rearrange("p (k h) -> p k h", k=KC),
                    wsrc.rearrange("(k p) h -> p k h", p=128)[
                        :, :, hc * 128:(hc + 1) * 128])
                tiles[(proj, hc)] = wt
        w_tiles[src] = tiles

    # ---------------- router + routing (scoped pools) ----------------
    with (
        tc.tile_pool(name="rt", bufs=1) as rt,
        tc.tile_pool(name="xs", bufs=2) as xs,
        tc.tile_pool(name="psl", bufs=NSC, space="PSUM") as psl,
        tc.tile_pool(name="pst", bufs=2, space="PSUM") as pst,
    ):
        scoresT = rt.tile([E, S], F32)
        pl = [psl.tile([E, 512], F32, tag="pl", name=f"pl{i}") for i in range(NSC)]
        for k in range(KC):
            xk = xs.tile([128, S], F32, tag="xk")
            eng = nc.sync if k % 2 == 0 else nc.gpsimd
            eng.dma_start(xk[:, :S // 2],
                          x_d[k * 128:(k + 1) * 128, :S // 2])
            eng.dma_start(xk[:, S // 2:],
                          x_d[k * 128:(k + 1) * 128, S // 2:])
            # fp32r rounding copy for the FFN path
            nc.vector.tensor_copy(x_r[:, k * S:(k + 1) * S], xk[:])
            for sc in range(NSC):
                nc.tensor.matmul(
                    pl[sc][:],
                    rw[:, k * E:(k + 1) * E],
                    xk[:, sc * 512:(sc + 1) * 512],
                    start=(k == 0), stop=(k == KC - 1),
                )
        load_w(0)
        for sc in range(NSC):
            nc.scalar.activation(scoresT[:, sc * 512:(sc + 1) * 512], pl[sc][:],
                                 AF.Sigmoid)

        # transpose scores -> [s, (t e)] layout
        scores = rt.tile([128, NT * E], F32)
        for t in range(NT):
            pt = pst.tile([128, E], F32, tag="pt")
            nc.tensor.transpose(pt[:], scoresT[:, t * 128:(t + 1) * 128],
                                ident[:E, :E])
            nc.vector.tensor_copy(scores[:, t * E:(t + 1) * E], pt[:])

        # ---- routing math (all DVE), layout [128, (t=16, e=16)] ----
        sb = rt.tile([128, NT * E], F32)
        bias_exp = rt.tile([128, E], F32)
        nc.gpsimd.partition_broadcast(bias_exp[:], bias_sb[0:1, :])
        sbv = sb.rearrange("p (t e) -> p t e", t=NT)
        scv = scores.rearrange("p (t e) -> p t e", t=NT)
        nc.vector.tensor_add(
            sbv, scv, bias_exp[:, None, :].to_broadcast([128, NT, E]))

        # group top-2 sum over each group of 4: max over the 6 pairwise sums
        sbg = sb.rearrange("p (t g j) -> p t g j", t=NT, g=G)
        t2s = rt.tile([128, NT * G], F32)
        t2sv = t2s.rearrange("p (t g) -> p t g", t=NT)
        tmp = rt.tile([128, NT * G], F32)
        tmpv = tmp.rearrange("p (t g) -> p t g", t=NT)
        pairs = [(a, b) for a in range(EPG) for b in range(a + 1, EPG)]
        first = True
        for (a, b) in pairs:
            dst = t2sv if first else tmpv
            nc.vector.tensor_add(dst, sbg[:, :, :, a], sbg[:, :, :, b])
            if not first:
                nc.vector.tensor_max(t2sv, t2sv, tmpv)
            first = False

        # second-largest group score per token: max over pairwise mins
        m2 = rt.tile([128, NT], F32)
        m2t = rt.tile([128, NT], F32)
        gpairs = [(a, b) for a in range(G) for b in range(a + 1, G)]
        first = True
        for (a, b) in gpairs:
            dst = m2 if first else m2t
            nc.vector.tensor_tensor(dst[:], t2sv[:, :, a], t2sv[:, :, b], OP.min)
            if not first:
                nc.vector.tensor_max(m2[:], m2[:], m2t[:])
            first = False

        # penalty: -1e30 on experts whose group is not in the top 2
        pen = rt.tile([128, NT * G], F32)
        penv = pen.rearrange("p (t g) -> p t g", t=NT)
        nc.vector.tensor_tensor(
            penv, t2sv, m2[:, :, None].to_broadcast([128, NT, G]), OP.is_lt)
        nc.vector.tensor_scalar_mul(pen[:], pen[:], -1e30)

        sbm = rt.tile([128, NT * E], F32)
        sbmg = sbm.rearrange("p (t g j) -> p t g j", t=NT, g=G)
        nc.vector.tensor_add(
            sbmg, sbg, penv[:, :, :, None].to_broadcast([128, NT, G, EPG]))

        # 4th largest of the masked biased scores per token -> threshold
        m8 = rt.tile([128, NT * 8], F32)
        for t in range(NT):
            nc.vector.max(m8[:, t * 8:(t + 1) * 8], sbm[:, t * E:(t + 1) * E])
        v4 = m8.rearrange("p (t k) -> p t k", t=NT)[:, :, TOPK - 1]

        msk = rt.tile([128, NT * E], F32)
        mskv = msk.rearrange("p (t e) -> p t e", t=NT)
        sbmv = sbm.rearrange("p (t e) -> p t e", t=NT)
        nc.vector.tensor_tensor(
            mskv, sbmv, v4[:, :, None].to_broadcast([128, NT, E]), OP.is_ge)

        # weights: unbiased scores at selected positions, renormalized
        wm = rt.tile([128, NT * E], F32)
        nc.vector.tensor_mul(wm[:], scores[:], msk[:])
        ws = rt.tile([128, NT], F32)
        nc.vector.reduce_sum(ws[:], wm.rearrange("p (t e) -> p t e", t=NT),
                             axis=mybir.AxisListType.X)
        nc.vector.tensor_scalar_add(ws[:], ws[:], 1e-20)
        wr = rt.tile([128, NT], F32)
        nc.vector.reciprocal(wr[:], ws[:])
        combv = comb.rearrange("p (t e) -> p t e", t=NT)
        nc.vector.tensor_mul(
            combv, wm.rearrange("p (t e) -> p t e", t=NT),
            wr[:, :, None].to_broadcast([128, NT, E]))

    # ---------------- FFN ----------------
    # down-proj weight loads (needed only in the down pass; emitted here so
    # they don't delay the x/router DMAs)
    for src in range(2):
        nc.sync.dma_start(
            wd[src].rearrange("p (hc c) -> p hc c", hc=NHC),
            dw_d[src].rearrange("(hc p) c -> p hc c", p=128))
    nc.sync.dma_start(
        wd[2].rearrange("p (hc c) -> p hc c", hc=NHC),
        sdw_d.rearrange("(hc p) c -> p hc c", p=128))

    cp = tc.alloc_tile_pool(name="cp", bufs=1)
    with (
        tc.tile_pool(name="cb", bufs=1) as cbp,
        tc.tile_pool(name="hsb", bufs=2) as hsb,
        tc.tile_pool(name="psg", bufs=3, space="PSUM") as psg,
        tc.tile_pool(name="psu", bufs=3, space="PSUM") as psu,
    ):
        combT = None
        for src in range(3):
            if src not in w_tiles:
                load_w(src)
            wt = w_tiles.pop(src)

            for hc in range(NHC):
                h_sb = hsb.tile([128, S], F32, tag="h")
                for sc in range(NSC):
                    pg = psg.tile([128, 512], F32, tag="pg")
                    pu = psu.tile([128, 512], F32, tag="pu")
                    for k in range(KC):
                        nc.tensor.matmul(
                            pg[:],
                            wt[("g", hc)][:, k * 128:(k + 1) * 128],
                            x_r[:, k * S + sc * 512: k * S + (sc + 1) * 512],
                            start=(k == 0), stop=(k == KC - 1))
                    for k in range(KC):
                        nc.tensor.matmul(
                            pu[:],
                            wt[("u", hc)][:, k * 128:(k + 1) * 128],
                            x_r[:, k * S + sc * 512: k * S + (sc + 1) * 512],
                            start=(k == 0), stop=(k == KC - 1))
                    sl = slice(sc * 512, (sc + 1) * 512)
                    nc.scalar.activation(h_sb[:, sl], pg[:], AF.Silu)
                    if src == 2:
                        # shared expert: no combine scaling; write f32r directly
                        nc.vector.tensor_mul(hw[src][hc][:, sl], h_sb[:, sl],
                                             pu[:])
                    else:
                        nc.vector.tensor_mul(h_sb[:, sl], h_sb[:, sl], pu[:])

                if src == 0 and combT is None:
                    # emit combine transposes after the first expert's g/u
                    # matmuls so the PE isn't stalled on the routing DVE chain
                    combT = cp.tile([E, S], F32R)
                    with tc.tile_pool(name="psct", bufs=2,
                                      space="PSUM") as psc:
                        for t in range(NT):
                            pct = psc.tile([E, 128], F32, tag="pct")
                            nc.tensor.transpose(
                                pct[:], comb[:, t * E:(t + 1) * E], ident[:])
                            nc.vector.tensor_copy(
                                combT[:, t * 128:(t + 1) * 128], pct[:])

                if src < 2 and hc == 0:
                    # broadcast this core's combine row across partitions by
                    # multiplying with a column-replicated one-hot (PE)
                    cb_exp = cbp.tile([128, S], F32, tag="cb", name="cb_exp")
                    with tc.tile_pool(name="pse2", bufs=2,
                                      space="PSUM") as pse2p:
                        for sc in range(NSC):
                            pe2 = pse2p.tile([128, 512], F32, tag="pe2")
                            nc.tensor.matmul(
                                pe2[:], esel[:, src * 128:(src + 1) * 128],
                                combT[:, sc * 512:(sc + 1) * 512],
                                start=True, stop=True)
                            nc.vector.tensor_copy(
                                cb_exp[:, sc * 512:(sc + 1) * 512], pe2[:])
                    cb_cur = cb_exp

                if src < 2:
                    nc.vector.tensor_mul(hw[src][hc][:], h_sb[:], cb_cur[:])

    cp.release()
    wp.release()
    xr_pool.release()

    # ---------------- down projection ----------------
    with (
        tc.tile_pool(name="oso", bufs=2) as oso,
        tc.tile_pool(name="pso", bufs=4, space="PSUM") as pso,
    ):
        for cc in range(NCC):
            os_t = oso.tile([128, S], F32, tag="os")
            for sc in range(NSC):
                po = pso.tile([128, 512], F32, tag="po")
                idx = 0
                for src in range(3):
                    for hc in range(NHC):
                        nc.tensor.matmul(
                            po[:],
                            wd[src][:, hc * C + cc * 128: hc * C + (cc + 1) * 128],
                            hw[src][hc][:, sc * 512:(sc + 1) * 512],
                            start=(idx == 0), stop=(idx == 5))
                        idx += 1
                nc.vector.tensor_copy(os_t[:, sc * 512:(sc + 1) * 512], po[:])
                if cc == NCC - 1:
                    nc.sync.dma_start(
                        out_d[cc * 128:(cc + 1) * 128,
                              sc * 512:(sc + 1) * 512],
                        os_t[:, sc * 512:(sc + 1) * 512])
            if cc < NCC - 1:
                nc.sync.dma_start(out_d[cc * 128:(cc + 1) * 128, :], os_t[:])

    hw_pool.release()
    consts.release()


_NC_CACHE = {}


def _get_nc():
    if "nc" not in _NC_CACHE:
        _NC_CACHE["nc"] = build()
    return _NC_CACHE["nc"]


def make_in_maps(x, router_w, correction_bias, gate_w, up_w, down_w,
                 shared_gate_w, shared_up_w, shared_down_w):
    x = np.asarray(x, dtype=np.float32)
    xT = np.ascontiguousarray(x.reshape(S, C).T)                 # [C, S]
    rwT = np.asarray(router_w, dtype=np.float32).T               # [C, E]
    rw_pk = np.ascontiguousarray(
        rwT.reshape(KC, 128, E).transpose(1, 0, 2).reshape(128, KC * E))
    bias = np.asarray(correction_bias, dtype=np.float32).reshape(1, E)
    sgT = np.asarray(shared_gate_w, dtype=np.float32).T          # [C, HS]
    suT = np.asarray(shared_up_w, dtype=np.float32).T            # [C, HS]
    sdT = np.asarray(shared_down_w, dtype=np.float32).T          # [HS, C]
    gate_w = np.asarray(gate_w, dtype=np.float32)
    up_w = np.asarray(up_w, dtype=np.float32)
    down_w = np.asarray(down_w, dtype=np.float32)

    in_maps = []
    for c in range(NCORES):
        es = slice(c * EPC, (c + 1) * EPC)
        hs = slice(c * HSL, (c + 1) * HSL)
        esel = np.zeros((E, EPC * 128), np.float32)
        esel[c * EPC, 0:128] = 1.0
        esel[c * EPC + 1, 128:256] = 1.0
        in_maps.append({
            "xT": xT,
            "rw": rw_pk,
            "bias": bias,
            "esel": esel,
            "gw": _round_f32r(gate_w[es]),
            "uw": _round_f32r(up_w[es]),
            "dw": _round_f32r(down_w[es]),
            "sgw": _round_f32r(sgT[:, hs]),
            "suw": _round_f32r(suT[:, hs]),
            "sdw": _round_f32r(sdT[hs, :]),
        })
    return in_maps


def kernel(x, router_w, correction_bias, gate_w, up_w, down_w,
           shared_gate_w, shared_up_w, shared_down_w):
    in_maps = make_in_maps(x, router_w, correction_bias, gate_w, up_w, down_w,
                           shared_gate_w, shared_up_w, shared_down_w)
    nc = _get_nc()
    res = run_bass_kernel_spmd(nc, in_maps, list(range(NCORES)))
    acc = np.zeros((C, S), np.float64)
    for c in range(NCORES):
        acc += res.results[c]["out"].astype(np.float64)
    return np.ascontiguousarray(acc.T).astype(np.float32).reshape(B, T, C)


# revision 18
# speedup vs baseline: 1.0968x; 1.0055x over previous
"""MoE FFN (grouped sigmoid top-k routing + shared expert) on 8 TRN2 NeuronCores.

Strategy: expert-parallel. Each core gets 2 of 16 routed experts plus 1/8 of
the shared expert (sharded along its hidden dim HS). x is replicated
(host-pre-transposed to [C, S] so every matmul contracts over the SBUF
partition dim). Routing is computed on-device, replicated on every core.
Each core emits a partial output [C, S]; the host sums the 8 partials and
transposes back.

dtypes: router matmuls run in full fp32 (top-k selection is sensitive to
input rounding); FFN matmuls run in fp32r (fp32 rounded to 11 mantissa bits,
full PE rate, ~1e-4 relative error).
"""

import numpy as np

import concourse.bacc as bacc
import concourse.mybir as mybir
from concourse import tile
from concourse.bass_utils import run_bass_kernel_spmd
from concourse.masks import make_identity

F32 = mybir.dt.float32
F32R = mybir.dt.float32r
AF = mybir.ActivationFunctionType
OP = mybir.AluOpType

# problem shapes (hardcoded; kernel.py must be self-contained)
B, T, C, H, HS = 2, 1024, 1024, 256, 2048
E, G, EPG = 16, 4, 4
TOPK = 4
NCORES = 8
S = B * T                  # 2048 tokens
EPC = E // NCORES          # 2 experts per core
HSL = HS // NCORES         # 256 shared-hidden rows per core
KC = C // 128              # 8 contraction chunks
NT = S // 128              # 16 token chunks
NSC = S // 512             # 4 moving (token) chunks of 512
NHC = H // 128             # 2 h chunks (same for HSL)
NCC = C // 128             # 8 output-row chunks


def _round_f32r(x: np.ndarray) -> np.ndarray:
    """Round fp32 to fp32r (RNE to 11 mantissa bits) — matches TRN2 PE."""
    u = np.ascontiguousarray(x, dtype=np.float32).view(np.uint32)
    u = u + 0x7FF + ((u >> 12) & 1)
    u = u & np.uint32(0xFFFFF000)
    return u.view(np.float32)


def build():
    nc = bacc.Bacc(
        "TRN2",
        target_bir_lowering=False,
        debug=False,
        enable_asserts=True,
        num_devices=NCORES,
    )
    # ---- DRAM I/O (per core) ----
    x_d = nc.declare_dram_parameter("xT", [C, S], F32, isOutput=False)
    rw_d = nc.declare_dram_parameter("rw", [128, 128], F32, isOutput=False)
    bias_d = nc.declare_dram_parameter("bias", [1, E], F32, isOutput=False)
    esel_d = nc.declare_dram_parameter("esel", [E, EPC * 128], F32R,
                                       isOutput=False)
    gw_d = nc.declare_dram_parameter("gw", [EPC, C, H], F32R, isOutput=False)
    uw_d = nc.declare_dram_parameter("uw", [EPC, C, H], F32R, isOutput=False)
    dw_d = nc.declare_dram_parameter("dw", [EPC, H, C], F32R, isOutput=False)
    sgw_d = nc.declare_dram_parameter("sgw", [C, HSL], F32R, isOutput=False)
    suw_d = nc.declare_dram_parameter("suw", [C, HSL], F32R, isOutput=False)
    sdw_d = nc.declare_dram_parameter("sdw", [HSL, C], F32R, isOutput=False)
    out_d = nc.declare_dram_parameter("out", [C, S], F32, isOutput=True)

    with tile.TileContext(nc) as tc:
        _emit(nc, tc, x_d, rw_d, bias_d, esel_d, gw_d, uw_d, dw_d,
              sgw_d, suw_d, sdw_d, out_d)
    nc.finalize()
    return nc


def _emit(nc, tc, x_d, rw_d, bias_d, esel_d, gw_d, uw_d, dw_d,
          sgw_d, suw_d, sdw_d, out_d):
    consts = tc.alloc_tile_pool(name="consts", bufs=1)
    ident = consts.tile([128, 128], F32)
    make_identity(nc, ident[:])
    rw = consts.tile([128, 128], F32)
    nc.sync.dma_start(rw[:], rw_d[:])
    bias_sb = consts.tile([1, E], F32)
    nc.sync.dma_start(bias_sb[:], bias_d[:])
    esel = consts.tile([E, EPC * 128], F32R)
    nc.sync.dma_start(esel[:], esel_d[:])
    # down-proj weights, resident (all 3 sources needed together in the
    # down pass): wd[src][p, hc*1024 + c] = dw[src][hc*128+p, c]
    wd = [consts.tile([128, NHC * C], F32R, tag=f"wd{i}", name=f"wd{i}")
          for i in range(3)]
    comb = consts.tile([128, NT * E], F32)       # combine weights [s, (t e)]

    # hw tiles [128, S] fp32r: (src, hc) -> silu(g)*u (* combine weight)
    hw_pool = tc.alloc_tile_pool(name="hw", bufs=1)
    hw = [[hw_pool.tile([128, S], F32R, tag=f"hw{src}{hc}",
                        name=f"hw{src}{hc}")
           for hc in range(NHC)] for src in range(3)]

    # x_r: fp32r copy of x, resident for all FFN matmuls
    xr_pool = tc.alloc_tile_pool(name="xr", bufs=1)
    x_r = xr_pool.tile([128, KC * S], F32R)

    # gate/up weight pool (opened early so expert 0's weights stream in
    # behind the first x chunk, during the router phase)
    wp = tc.alloc_tile_pool(name="wp", bufs=2)
    w_tiles = {}

    def load_w(src):
        # one [128, KC*128] tile per (proj, hc): finer slot rotation lets the
        # next source's first-half weights stream while the current source is
        # still computing its second half
        tiles = {}
        for proj, wsrc in (("g", gw_d[src] if src < 2 else sgw_d),
                           ("u", uw_d[src] if src < 2 else suw_d)):
            for hc in range(NHC):
                wt = wp.tile([128, KC * 128], F32R, tag=f"{proj}{hc}",
                             name=f"w{proj}{src}{hc}")
                nc.sync.dma_start(
                    wt.rearrange("p (k h) -> p k h", k=KC),
                    wsrc.rearrange("(k p) h -> p k h", p=128)[
                        :, :, hc * 128:(hc + 1) * 128])
                tiles[(proj, hc)] = wt
        w_tiles[src] = tiles

    # ---------------- router + routing (scoped pools) ----------------
    with (
        tc.tile_pool(name="rt", bufs=1) as rt,
        tc.tile_pool(name="xs", bufs=2) as xs,
        tc.tile_pool(name="psl", bufs=NSC, space="PSUM") as psl,
        tc.tile_pool(name="pst", bufs=2, space="PSUM") as pst,
    ):
        scoresT = rt.tile([E, S], F32)
        pl = [psl.tile([E, 512], F32, tag="pl", name=f"pl{i}") for i in range(NSC)]
        for k in range(KC):
            xk = xs.tile([128, S], F32, tag="xk")
            eng = nc.sync if k % 2 == 0 else nc.gpsimd
            if k == 0:
                nc.sync.dma_start(xk[:, :512], x_d[:128, :512])
                nc.gpsimd.dma_start(xk[:, 512:1024], x_d[:128, 512:1024])
                nc.sync.dma_start(xk[:, 1024:], x_d[:128, 1024:])
            else:
                eng.dma_start(xk[:, :S // 2],
                              x_d[k * 128:(k + 1) * 128, :S // 2])
                eng.dma_start(xk[:, S // 2:],
                              x_d[k * 128:(k + 1) * 128, S // 2:])
            # fp32r rounding copy for the FFN path
            nc.vector.tensor_copy(x_r[:, k * S:(k + 1) * S], xk[:])
            for sc in range(NSC):
                nc.tensor.matmul(
                    pl[sc][:],
                    rw[:, k * E:(k + 1) * E],
                    xk[:, sc * 512:(sc + 1) * 512],
                    start=(k == 0), stop=(k == KC - 1),
                )
        load_w(0)
        for sc in range(NSC):
            nc.scalar.activation(scoresT[:, sc * 512:(sc + 1) * 512], pl[sc][:],
                                 AF.Sigmoid)

        # transpose scores -> [s, (t e)] layout
        scores = rt.tile([128, NT * E], F32)
        for t in range(NT):
            pt = pst.tile([128, E], F32, tag="pt")
            nc.tensor.transpose(pt[:], scoresT[:, t * 128:(t + 1) * 128],
                                ident[:E, :E])
            nc.vector.tensor_copy(scores[:, t * E:(t + 1) * E], pt[:])

        # ---- routing math (all DVE), layout [128, (t=16, e=16)] ----
        sb = rt.tile([128, NT * E], F32)
        bias_exp = rt.tile([128, E], F32)
        nc.gpsimd.partition_broadcast(bias_exp[:], bias_sb[0:1, :])
        sbv = sb.rearrange("p (t e) -> p t e", t=NT)
        scv = scores.rearrange("p (t e) -> p t e", t=NT)
        nc.vector.tensor_add(
            sbv, scv, bias_exp[:, None, :].to_broadcast([128, NT, E]))

        # group top-2 sum over each group of 4: max over the 6 pairwise sums
        sbg = sb.rearrange("p (t g j) -> p t g j", t=NT, g=G)
        t2s = rt.tile([128, NT * G], F32)
        t2sv = t2s.rearrange("p (t g) -> p t g", t=NT)
        tmp = rt.tile([128, NT * G], F32)
        tmpv = tmp.rearrange("p (t g) -> p t g", t=NT)
        pairs = [(a, b) for a in range(EPG) for b in range(a + 1, EPG)]
        first = True
        for (a, b) in pairs:
            dst = t2sv if first else tmpv
            nc.vector.tensor_add(dst, sbg[:, :, :, a], sbg[:, :, :, b])
            if not first:
                nc.vector.tensor_max(t2sv, t2sv, tmpv)
            first = False

        # second-largest group score per token: max over pairwise mins
        m2 = rt.tile([128, NT], F32)
        m2t = rt.tile([128, NT], F32)
        gpairs = [(a, b) for a in range(G) for b in range(a + 1, G)]
        first = True
        for (a, b) in gpairs:
            dst = m2 if first else m2t
            nc.vector.tensor_tensor(dst[:], t2sv[:, :, a], t2sv[:, :, b], OP.min)
            if not first:
                nc.vector.tensor_max(m2[:], m2[:], m2t[:])
            first = False

        # penalty: -1e30 on experts whose group is not in the top 2
        pen = rt.tile([128, NT * G], F32)
        penv = pen.rearrange("p (t g) -> p t g", t=NT)
        nc.vector.tensor_tensor(
            penv, t2sv, m2[:, :, None].to_broadcast([128, NT, G]), OP.is_lt)
        nc.vector.tensor_scalar_mul(pen[:], pen[:], -1e30)

        sbm = rt.tile([128, NT * E], F32)
        sbmg = sbm.rearrange("p (t g j) -> p t g j", t=NT, g=G)
        nc.vector.tensor_add(
            sbmg, sbg, penv[:, :, :, None].to_broadcast([128, NT, G, EPG]))

        # 4th largest of the masked biased scores per token -> threshold
        m8 = rt.tile([128, NT * 8], F32)
        for t in range(NT):
            nc.vector.max(m8[:, t * 8:(t + 1) * 8], sbm[:, t * E:(t + 1) * E])
        v4 = m8.rearrange("p (t k) -> p t k", t=NT)[:, :, TOPK - 1]

        msk = rt.tile([128, NT * E], F32)
        mskv = msk.rearrange("p (t e) -> p t e", t=NT)
        sbmv = sbm.rearrange("p (t e) -> p t e", t=NT)
        nc.vector.tensor_tensor(
            mskv, sbmv, v4[:, :, None].to_broadcast([128, NT, E]), OP.is_ge)

        # weights: unbiased scores at selected positions, renormalized
        wm = rt.tile([128, NT * E], F32)
        nc.vector.tensor_mul(wm[:], scores[:], msk[:])
        ws = rt.tile([128, NT], F32)
        nc.vector.reduce_sum(ws[:], wm.rearrange("p (t e) -> p t e", t=NT),
                             axis=mybir.AxisListType.X)
        nc.vector.tensor_scalar_add(ws[:], ws[:], 1e-20)
        wr = rt.tile([128, NT], F32)
        nc.vector.reciprocal(wr[:], ws[:])
        combv = comb.rearrange("p (t e) -> p t e", t=NT)
        nc.vector.tensor_mul(
            combv, wm.rearrange("p (t e) -> p t e", t=NT),
            wr[:, :, None].to_broadcast([128, NT, E]))

    # ---------------- FFN ----------------
    # down-proj weight loads (needed only in the down pass; emitted here so
    # they don't delay the x/router DMAs)
    for src in range(2):
        nc.sync.dma_start(
            wd[src].rearrange("p (hc c) -> p hc c", hc=NHC),
            dw_d[src].rearrange("(hc p) c -> p hc c", p=128))
    nc.sync.dma_start(
        wd[2].rearrange("p (hc c) -> p hc c", hc=NHC),
        sdw_d.rearrange("(hc p) c -> p hc c", p=128))

    cp = tc.alloc_tile_pool(name="cp", bufs=1)
    with (
        tc.tile_pool(name="cb", bufs=1) as cbp,
        tc.tile_pool(name="hsb", bufs=2) as hsb,
        tc.tile_pool(name="psg", bufs=3, space="PSUM") as psg,
        tc.tile_pool(name="psu", bufs=3, space="PSUM") as psu,
    ):
        combT = None
        for src in range(3):
            if src not in w_tiles:
                load_w(src)
            wt = w_tiles.pop(src)

            for hc in range(NHC):
                h_sb = hsb.tile([128, S], F32, tag="h")
                for sc in range(NSC):
                    pg = psg.tile([128, 512], F32, tag="pg")
                    pu = psu.tile([128, 512], F32, tag="pu")
                    for k in range(KC):
                        nc.tensor.matmul(
                            pg[:],
                            wt[("g", hc)][:, k * 128:(k + 1) * 128],
                            x_r[:, k * S + sc * 512: k * S + (sc + 1) * 512],
                            start=(k == 0), stop=(k == KC - 1))
                    for k in range(KC):
                        nc.tensor.matmul(
                            pu[:],
                            wt[("u", hc)][:, k * 128:(k + 1) * 128],
                            x_r[:, k * S + sc * 512: k * S + (sc + 1) * 512],
                            start=(k == 0), stop=(k == KC - 1))
                    sl = slice(sc * 512, (sc + 1) * 512)
                    nc.scalar.activation(h_sb[:, sl], pg[:], AF.Silu)
                    if src == 2:
                        # shared expert: no combine scaling; write f32r directly
                        nc.vector.tensor_mul(hw[src][hc][:, sl], h_sb[:, sl],
                                             pu[:])
                    else:
                        nc.vector.tensor_mul(h_sb[:, sl], h_sb[:, sl], pu[:])

                if src == 0 and combT is None:
                    # emit combine transposes after the first expert's g/u
                    # matmuls so the PE isn't stalled on the routing DVE chain
                    combT = cp.tile([E, S], F32R)
                    with tc.tile_pool(name="psct", bufs=2,
                                      space="PSUM") as psc:
                        for t in range(NT):
                            pct = psc.tile([E, 128], F32, tag="pct")
                            nc.tensor.transpose(
                                pct[:], comb[:, t * E:(t + 1) * E], ident[:])
                            nc.vector.tensor_copy(
                                combT[:, t * 128:(t + 1) * 128], pct[:])

                if src < 2 and hc == 0:
                    # broadcast this core's combine row across partitions by
                    # multiplying with a column-replicated one-hot (PE)
                    cb_exp = cbp.tile([128, S], F32, tag="cb", name="cb_exp")
                    with tc.tile_pool(name="pse2", bufs=2,
                                      space="PSUM") as pse2p:
                        for sc in range(NSC):
                            pe2 = pse2p.tile([128, 512], F32, tag="pe2")
                            nc.tensor.matmul(
                                pe2[:], esel[:, src * 128:(src + 1) * 128],
                                combT[:, sc * 512:(sc + 1) * 512],
                                start=True, stop=True)
                            nc.vector.tensor_copy(
                                cb_exp[:, sc * 512:(sc + 1) * 512], pe2[:])
                    cb_cur = cb_exp

                if src < 2:
                    nc.vector.tensor_mul(hw[src][hc][:], h_sb[:], cb_cur[:])

    cp.release()
    wp.release()
    xr_pool.release()

    # ---------------- down projection ----------------
    with (
        tc.tile_pool(name="oso", bufs=2) as oso,
        tc.tile_pool(name="pso", bufs=4, space="PSUM") as pso,
    ):
        for cc in range(NCC):
            os_t = oso.tile([128, S], F32, tag="os")
            for sc in range(NSC):
                po = pso.tile([128, 512], F32, tag="po")
                idx = 0
                for src in range(3):
                    for hc in range(NHC):
                        nc.tensor.matmul(
                            po[:],
                            wd[src][:, hc * C + cc * 128: hc * C + (cc + 1) * 128],
                            hw[src][hc][:, sc * 512:(sc + 1) * 512],
                            start=(idx == 0), stop=(idx == 5))
                        idx += 1
                nc.vector.tensor_copy(os_t[:, sc * 512:(sc + 1) * 512], po[:])
                if cc == NCC - 1:
                    oeng = nc.sync if sc % 2 == 0 else nc.gpsimd
                    oeng.dma_start(
                        out_d[cc * 128:(cc + 1) * 128,
                              sc * 512:(sc + 1) * 512],
                        os_t[:, sc * 512:(sc + 1) * 512])
            if cc < NCC - 1:
                nc.sync.dma_start(out_d[cc * 128:(cc + 1) * 128, :], os_t[:])

    hw_pool.release()
    consts.release()


_NC_CACHE = {}


def _get_nc():
    if "nc" not in _NC_CACHE:
        _NC_CACHE["nc"] = build()
    return _NC_CACHE["nc"]


def make_in_maps(x, router_w, correction_bias, gate_w, up_w, down_w,
                 shared_gate_w, shared_up_w, shared_down_w):
    x = np.asarray(x, dtype=np.float32)
    xT = np.ascontiguousarray(x.reshape(S, C).T)                 # [C, S]
    rwT = np.asarray(router_w, dtype=np.float32).T               # [C, E]
    rw_pk = np.ascontiguousarray(
        rwT.reshape(KC, 128, E).transpose(1, 0, 2).reshape(128, KC * E))
    bias = np.asarray(correction_bias, dtype=np.float32).reshape(1, E)
    sgT = np.asarray(shared_gate_w, dtype=np.float32).T          # [C, HS]
    suT = np.asarray(shared_up_w, dtype=np.float32).T            # [C, HS]
    sdT = np.asarray(shared_down_w, dtype=np.float32).T          # [HS, C]
    gate_w = np.asarray(gate_w, dtype=np.float32)
    up_w = np.asarray(up_w, dtype=np.float32)
    down_w = np.asarray(down_w, dtype=np.float32)

    in_maps = []
    for c in range(NCORES):
        es = slice(c * EPC, (c + 1) * EPC)
        hs = slice(c * HSL, (c + 1) * HSL)
        esel = np.zeros((E, EPC * 128), np.float32)
        esel[c * EPC, 0:128] = 1.0
        esel[c * EPC + 1, 128:256] = 1.0
        in_maps.append({
            "xT": xT,
            "rw": rw_pk,
            "bias": bias,
            "esel": esel,
            "gw": _round_f32r(gate_w[es]),
            "uw": _round_f32r(up_w[es]),
            "dw": _round_f32r(down_w[es]),
            "sgw": _round_f32r(sgT[:, hs]),
            "suw": _round_f32r(suT[:, hs]),
            "sdw": _round_f32r(sdT[hs, :]),
        })
    return in_maps


def kernel(x, router_w, correction_bias, gate_w, up_w, down_w,
           shared_gate_w, shared_up_w, shared_down_w):
    in_maps = make_in_maps(x, router_w, correction_bias, gate_w, up_w, down_w,
                           shared_gate_w, shared_up_w, shared_down_w)
    nc = _get_nc()
    res = run_bass_kernel_spmd(nc, in_maps, list(range(NCORES)))
    acc = np.zeros((C, S), np.float64)
    for c in range(NCORES):
        acc += res.results[c]["out"].astype(np.float64)
    return np.ascontiguousarray(acc.T).astype(np.float32).reshape(B, T, C)


# revision 19
# speedup vs baseline: 1.1158x; 1.0172x over previous
"""MoE FFN (grouped sigmoid top-k routing + shared expert) on 8 TRN2 NeuronCores.

Strategy: expert-parallel. Each core gets 2 of 16 routed experts plus 1/8 of
the shared expert (sharded along its hidden dim HS). x is replicated
(host-pre-transposed to [C, S] so every matmul contracts over the SBUF
partition dim). Routing is computed on-device, replicated on every core.
Each core emits a partial output [C, S]; the host sums the 8 partials and
transposes back.

dtypes: router matmuls run in full fp32 (top-k selection is sensitive to
input rounding); FFN matmuls run in fp32r (fp32 rounded to 11 mantissa bits,
full PE rate, ~1e-4 relative error).
"""

import numpy as np

import concourse.bacc as bacc
import concourse.mybir as mybir
from concourse import tile
from concourse.bass_utils import run_bass_kernel_spmd
from concourse.masks import make_identity

F32 = mybir.dt.float32
F32R = mybir.dt.float32r
AF = mybir.ActivationFunctionType
OP = mybir.AluOpType

# problem shapes (hardcoded; kernel.py must be self-contained)
B, T, C, H, HS = 2, 1024, 1024, 256, 2048
E, G, EPG = 16, 4, 4
TOPK = 4
NCORES = 8
S = B * T                  # 2048 tokens
EPC = E // NCORES          # 2 experts per core
HSL = HS // NCORES         # 256 shared-hidden rows per core
KC = C // 128              # 8 contraction chunks
NT = S // 128              # 16 token chunks
NSC = S // 512             # 4 moving (token) chunks of 512
NHC = H // 128             # 2 h chunks (same for HSL)
NCC = C // 128             # 8 output-row chunks


def _round_f32r(x: np.ndarray) -> np.ndarray:
    """Round fp32 to fp32r (RNE to 11 mantissa bits) — matches TRN2 PE."""
    u = np.ascontiguousarray(x, dtype=np.float32).view(np.uint32)
    u = u + 0x7FF + ((u >> 12) & 1)
    u = u & np.uint32(0xFFFFF000)
    return u.view(np.float32)


def build():
    nc = bacc.Bacc(
        "TRN2",
        target_bir_lowering=False,
        debug=False,
        enable_asserts=True,
        num_devices=NCORES,
    )
    # ---- DRAM I/O (per core) ----
    x_d = nc.declare_dram_parameter("xT", [C, S], F32, isOutput=False)
    rw_d = nc.declare_dram_parameter("rw", [128, 128], F32, isOutput=False)
    bias_d = nc.declare_dram_parameter("bias", [1, E], F32, isOutput=False)
    esel_d = nc.declare_dram_parameter("esel", [E, EPC * 128], F32R,
                                       isOutput=False)
    gw_d = nc.declare_dram_parameter("gw", [EPC, C, H], F32R, isOutput=False)
    uw_d = nc.declare_dram_parameter("uw", [EPC, C, H], F32R, isOutput=False)
    dw_d = nc.declare_dram_parameter("dw", [EPC, H, C], F32R, isOutput=False)
    sgw_d = nc.declare_dram_parameter("sgw", [C, HSL], F32R, isOutput=False)
    suw_d = nc.declare_dram_parameter("suw", [C, HSL], F32R, isOutput=False)
    sdw_d = nc.declare_dram_parameter("sdw", [HSL, C], F32R, isOutput=False)
    out_d = nc.declare_dram_parameter("out", [C, S], F32, isOutput=True)

    with tile.TileContext(nc) as tc:
        _emit(nc, tc, x_d, rw_d, bias_d, esel_d, gw_d, uw_d, dw_d,
              sgw_d, suw_d, sdw_d, out_d)
    nc.finalize()
    return nc


def _emit(nc, tc, x_d, rw_d, bias_d, esel_d, gw_d, uw_d, dw_d,
          sgw_d, suw_d, sdw_d, out_d):
    consts = tc.alloc_tile_pool(name="consts", bufs=1)
    ident = consts.tile([128, 128], F32)
    make_identity(nc, ident[:])
    rw = consts.tile([128, 128], F32)
    nc.sync.dma_start(rw[:], rw_d[:])
    bias_sb = consts.tile([1, E], F32)
    nc.sync.dma_start(bias_sb[:], bias_d[:])
    esel = consts.tile([E, EPC * 128], F32R)
    nc.sync.dma_start(esel[:], esel_d[:])
    # down-proj weights, resident (all 3 sources needed together in the
    # down pass): wd[src][p, hc*1024 + c] = dw[src][hc*128+p, c]
    wd = [consts.tile([128, NHC * C], F32R, tag=f"wd{i}", name=f"wd{i}")
          for i in range(3)]
    comb = consts.tile([128, NT * E], F32)       # combine weights [s, (t e)]

    # hw tiles [128, S] fp32r: (src, hc) -> silu(g)*u (* combine weight)
    hw_pool = tc.alloc_tile_pool(name="hw", bufs=1)
    hw = [[hw_pool.tile([128, S], F32R, tag=f"hw{src}{hc}",
                        name=f"hw{src}{hc}")
           for hc in range(NHC)] for src in range(3)]

    # x_r: fp32r copy of x, resident for all FFN matmuls
    xr_pool = tc.alloc_tile_pool(name="xr", bufs=1)
    x_r = xr_pool.tile([128, KC * S], F32R)

    # gate/up weight pool (opened early so expert 0's weights stream in
    # behind the first x chunk, during the router phase)
    wp = tc.alloc_tile_pool(name="wp", bufs=2)
    w_tiles = {}

    def load_w(src):
        # one [128, KC*128] tile per (proj, hc): finer slot rotation lets the
        # next source's first-half weights stream while the current source is
        # still computing its second half
        tiles = {}
        for proj, wsrc in (("g", gw_d[src] if src < 2 else sgw_d),
                           ("u", uw_d[src] if src < 2 else suw_d)):
            for hc in range(NHC):
                wt = wp.tile([128, KC * 128], F32R, tag=f"{proj}{hc}",
                             name=f"w{proj}{src}{hc}")
                nc.sync.dma_start(
                    wt.rearrange("p (k h) -> p k h", k=KC),
                    wsrc.rearrange("(k p) h -> p k h", p=128)[
                        :, :, hc * 128:(hc + 1) * 128])
                tiles[(proj, hc)] = wt
        w_tiles[src] = tiles

    # ---------------- router + routing (scoped pools) ----------------
    with (
        tc.tile_pool(name="rt", bufs=1) as rt,
        tc.tile_pool(name="xs", bufs=2) as xs,
        tc.tile_pool(name="psl", bufs=NSC, space="PSUM") as psl,
        tc.tile_pool(name="pst", bufs=2, space="PSUM") as pst,
    ):
        scoresT = rt.tile([E, S], F32)
        pl = [psl.tile([E, 512], F32, tag="pl", name=f"pl{i}") for i in range(NSC)]
        HS2 = S // 2
        for k in range(KC):
            # two half-chunk tiles with separate tags: the WAR on slot reuse
            # releases per half, so the DMA stream runs ahead of the PE
            xlo = xs.tile([128, HS2], F32, tag="xkl", name="xlo")
            xhi = xs.tile([128, HS2], F32, tag="xkh", name="xhi")
            eng = nc.sync if k % 2 == 0 else nc.gpsimd
            oth = nc.gpsimd if k % 2 == 0 else nc.sync
            if k == 0:
                nc.sync.dma_start(xlo[:, :512], x_d[:128, :512])
                nc.gpsimd.dma_start(xlo[:, 512:], x_d[:128, 512:HS2])
                nc.sync.dma_start(xhi[:], x_d[:128, HS2:])
            else:
                eng.dma_start(xlo[:], x_d[k * 128:(k + 1) * 128, :HS2])
                oth.dma_start(xhi[:], x_d[k * 128:(k + 1) * 128, HS2:])
            # fp32r rounding copies for the FFN path
            nc.vector.tensor_copy(x_r[:, k * S:k * S + HS2], xlo[:])
            nc.vector.tensor_copy(x_r[:, k * S + HS2:(k + 1) * S], xhi[:])
            for sc in range(NSC):
                src_t = xlo if sc < 2 else xhi
                nc.tensor.matmul(
                    pl[sc][:],
                    rw[:, k * E:(k + 1) * E],
                    src_t[:, (sc % 2) * 512:(sc % 2 + 1) * 512],
                    start=(k == 0), stop=(k == KC - 1),
                )
        load_w(0)
        for sc in range(NSC):
            nc.scalar.activation(scoresT[:, sc * 512:(sc + 1) * 512], pl[sc][:],
                                 AF.Sigmoid)

        # transpose scores -> [s, (t e)] layout
        scores = rt.tile([128, NT * E], F32)
        for t in range(NT):
            pt = pst.tile([128, E], F32, tag="pt")
            nc.tensor.transpose(pt[:], scoresT[:, t * 128:(t + 1) * 128],
                                ident[:E, :E])
            nc.vector.tensor_copy(scores[:, t * E:(t + 1) * E], pt[:])

        # ---- routing math (all DVE), layout [128, (t=16, e=16)] ----
        sb = rt.tile([128, NT * E], F32)
        bias_exp = rt.tile([128, E], F32)
        nc.gpsimd.partition_broadcast(bias_exp[:], bias_sb[0:1, :])
        sbv = sb.rearrange("p (t e) -> p t e", t=NT)
        scv = scores.rearrange("p (t e) -> p t e", t=NT)
        nc.vector.tensor_add(
            sbv, scv, bias_exp[:, None, :].to_broadcast([128, NT, E]))

        # group top-2 sum over each group of 4: max over the 6 pairwise sums
        sbg = sb.rearrange("p (t g j) -> p t g j", t=NT, g=G)
        t2s = rt.tile([128, NT * G], F32)
        t2sv = t2s.rearrange("p (t g) -> p t g", t=NT)
        tmp = rt.tile([128, NT * G], F32)
        tmpv = tmp.rearrange("p (t g) -> p t g", t=NT)
        pairs = [(a, b) for a in range(EPG) for b in range(a + 1, EPG)]
        first = True
        for (a, b) in pairs:
            dst = t2sv if first else tmpv
            nc.vector.tensor_add(dst, sbg[:, :, :, a], sbg[:, :, :, b])
            if not first:
                nc.vector.tensor_max(t2sv, t2sv, tmpv)
            first = False

        # second-largest group score per token: max over pairwise mins
        m2 = rt.tile([128, NT], F32)
        m2t = rt.tile([128, NT], F32)
        gpairs = [(a, b) for a in range(G) for b in range(a + 1, G)]
        first = True
        for (a, b) in gpairs:
            dst = m2 if first else m2t
            nc.vector.tensor_tensor(dst[:], t2sv[:, :, a], t2sv[:, :, b], OP.min)
            if not first:
                nc.vector.tensor_max(m2[:], m2[:], m2t[:])
            first = False

        # penalty: -1e30 on experts whose group is not in the top 2
        pen = rt.tile([128, NT * G], F32)
        penv = pen.rearrange("p (t g) -> p t g", t=NT)
        nc.vector.tensor_tensor(
            penv, t2sv, m2[:, :, None].to_broadcast([128, NT, G]), OP.is_lt)
        nc.vector.tensor_scalar_mul(pen[:], pen[:], -1e30)

        sbm = rt.tile([128, NT * E], F32)
        sbmg = sbm.rearrange("p (t g j) -> p t g j", t=NT, g=G)
        nc.vector.tensor_add(
            sbmg, sbg, penv[:, :, :, None].to_broadcast([128, NT, G, EPG]))

        # 4th largest of the masked biased scores per token -> threshold
        m8 = rt.tile([128, NT * 8], F32)
        for t in range(NT):
            nc.vector.max(m8[:, t * 8:(t + 1) * 8], sbm[:, t * E:(t + 1) * E])
        v4 = m8.rearrange("p (t k) -> p t k", t=NT)[:, :, TOPK - 1]

        msk = rt.tile([128, NT * E], F32)
        mskv = msk.rearrange("p (t e) -> p t e", t=NT)
        sbmv = sbm.rearrange("p (t e) -> p t e", t=NT)
        nc.vector.tensor_tensor(
            mskv, sbmv, v4[:, :, None].to_broadcast([128, NT, E]), OP.is_ge)

        # weights: unbiased scores at selected positions, renormalized
        wm = rt.tile([128, NT * E], F32)
        nc.vector.tensor_mul(wm[:], scores[:], msk[:])
        ws = rt.tile([128, NT], F32)
        nc.vector.reduce_sum(ws[:], wm.rearrange("p (t e) -> p t e", t=NT),
                             axis=mybir.AxisListType.X)
        nc.vector.tensor_scalar_add(ws[:], ws[:], 1e-20)
        wr = rt.tile([128, NT], F32)
        nc.vector.reciprocal(wr[:], ws[:])
        combv = comb.rearrange("p (t e) -> p t e", t=NT)
        nc.vector.tensor_mul(
            combv, wm.rearrange("p (t e) -> p t e", t=NT),
            wr[:, :, None].to_broadcast([128, NT, E]))

    # ---------------- FFN ----------------
    # down-proj weight loads (needed only in the down pass; emitted here so
    # they don't delay the x/router DMAs)
    for src in range(2):
        nc.sync.dma_start(
            wd[src].rearrange("p (hc c) -> p hc c", hc=NHC),
            dw_d[src].rearrange("(hc p) c -> p hc c", p=128))
    nc.sync.dma_start(
        wd[2].rearrange("p (hc c) -> p hc c", hc=NHC),
        sdw_d.rearrange("(hc p) c -> p hc c", p=128))

    cp = tc.alloc_tile_pool(name="cp", bufs=1)
    with (
        tc.tile_pool(name="cb", bufs=1) as cbp,
        tc.tile_pool(name="hsb", bufs=2) as hsb,
        tc.tile_pool(name="psg", bufs=3, space="PSUM") as psg,
        tc.tile_pool(name="psu", bufs=3, space="PSUM") as psu,
    ):
        combT = None
        for src in range(3):
            if src not in w_tiles:
                load_w(src)
            wt = w_tiles.pop(src)

            for hc in range(NHC):
                h_sb = hsb.tile([128, S], F32, tag="h")
                for sc in range(NSC):
                    pg = psg.tile([128, 512], F32, tag="pg")
                    pu = psu.tile([128, 512], F32, tag="pu")
                    for k in range(KC):
                        nc.tensor.matmul(
                            pg[:],
                            wt[("g", hc)][:, k * 128:(k + 1) * 128],
                            x_r[:, k * S + sc * 512: k * S + (sc + 1) * 512],
                            start=(k == 0), stop=(k == KC - 1))
                    for k in range(KC):
                        nc.tensor.matmul(
                            pu[:],
                            wt[("u", hc)][:, k * 128:(k + 1) * 128],
                            x_r[:, k * S + sc * 512: k * S + (sc + 1) * 512],
                            start=(k == 0), stop=(k == KC - 1))
                    sl = slice(sc * 512, (sc + 1) * 512)
                    nc.scalar.activation(h_sb[:, sl], pg[:], AF.Silu)
                    if src == 2:
                        # shared expert: no combine scaling; write f32r directly
                        nc.vector.tensor_mul(hw[src][hc][:, sl], h_sb[:, sl],
                                             pu[:])
                    else:
                        nc.vector.tensor_mul(h_sb[:, sl], h_sb[:, sl], pu[:])

                if src == 0 and combT is None:
                    # emit combine transposes after the first expert's g/u
                    # matmuls so the PE isn't stalled on the routing DVE chain
                    combT = cp.tile([E, S], F32R)
                    with tc.tile_pool(name="psct", bufs=2,
                                      space="PSUM") as psc:
                        for t in range(NT):
                            pct = psc.tile([E, 128], F32, tag="pct")
                            nc.tensor.transpose(
                                pct[:], comb[:, t * E:(t + 1) * E], ident[:])
                            nc.vector.tensor_copy(
                                combT[:, t * 128:(t + 1) * 128], pct[:])

                if src < 2 and hc == 0:
                    # broadcast this core's combine row across partitions by
                    # multiplying with a column-replicated one-hot (PE)
                    cb_exp = cbp.tile([128, S], F32, tag="cb", name="cb_exp")
                    with tc.tile_pool(name="pse2", bufs=2,
                                      space="PSUM") as pse2p:
                        for sc in range(NSC):
                            pe2 = pse2p.tile([128, 512], F32, tag="pe2")
                            nc.tensor.matmul(
                                pe2[:], esel[:, src * 128:(src + 1) * 128],
                                combT[:, sc * 512:(sc + 1) * 512],
                                start=True, stop=True)
                            nc.vector.tensor_copy(
                                cb_exp[:, sc * 512:(sc + 1) * 512], pe2[:])
                    cb_cur = cb_exp

                if src < 2:
                    nc.vector.tensor_mul(hw[src][hc][:], h_sb[:], cb_cur[:])

    cp.release()
    wp.release()
    xr_pool.release()

    # ---------------- down projection ----------------
    with (
        tc.tile_pool(name="oso", bufs=2) as oso,
        tc.tile_pool(name="pso", bufs=4, space="PSUM") as pso,
    ):
        for cc in range(NCC):
            os_t = oso.tile([128, S], F32, tag="os")
            for sc in range(NSC):
                po = pso.tile([128, 512], F32, tag="po")
                idx = 0
                for src in range(3):
                    for hc in range(NHC):
                        nc.tensor.matmul(
                            po[:],
                            wd[src][:, hc * C + cc * 128: hc * C + (cc + 1) * 128],
                            hw[src][hc][:, sc * 512:(sc + 1) * 512],
                            start=(idx == 0), stop=(idx == 5))
                        idx += 1
                nc.vector.tensor_copy(os_t[:, sc * 512:(sc + 1) * 512], po[:])
                if cc == NCC - 1:
                    oeng = nc.sync if sc % 2 == 0 else nc.gpsimd
                    oeng.dma_start(
                        out_d[cc * 128:(cc + 1) * 128,
                              sc * 512:(sc + 1) * 512],
                        os_t[:, sc * 512:(sc + 1) * 512])
            if cc < NCC - 1:
                nc.sync.dma_start(out_d[cc * 128:(cc + 1) * 128, :], os_t[:])

    hw_pool.release()
    consts.release()


_NC_CACHE = {}


def _get_nc():
    if "nc" not in _NC_CACHE:
        _NC_CACHE["nc"] = build()
    return _NC_CACHE["nc"]


def make_in_maps(x, router_w, correction_bias, gate_w, up_w, down_w,
                 shared_gate_w, shared_up_w, shared_down_w):
    x = np.asarray(x, dtype=np.float32)
    xT = np.ascontiguousarray(x.reshape(S, C).T)                 # [C, S]
    rwT = np.asarray(router_w, dtype=np.float32).T               # [C, E]
    rw_pk = np.ascontiguousarray(
        rwT.reshape(KC, 128, E).transpose(1, 0, 2).reshape(128, KC * E))
    bias = np.asarray(correction_bias, dtype=np.float32).reshape(1, E)
    sgT = np.asarray(shared_gate_w, dtype=np.float32).T          # [C, HS]
    suT = np.asarray(shared_up_w, dtype=np.float32).T            # [C, HS]
    sdT = np.asarray(shared_down_w, dtype=np.float32).T          # [HS, C]
    gate_w = np.asarray(gate_w, dtype=np.float32)
    up_w = np.asarray(up_w, dtype=np.float32)
    down_w = np.asarray(down_w, dtype=np.float32)

    in_maps = []
    for c in range(NCORES):
        es = slice(c * EPC, (c + 1) * EPC)
        hs = slice(c * HSL, (c + 1) * HSL)
        esel = np.zeros((E, EPC * 128), np.float32)
        esel[c * EPC, 0:128] = 1.0
        esel[c * EPC + 1, 128:256] = 1.0
        in_maps.append({
            "xT": xT,
            "rw": rw_pk,
            "bias": bias,
            "esel": esel,
            "gw": _round_f32r(gate_w[es]),
            "uw": _round_f32r(up_w[es]),
            "dw": _round_f32r(down_w[es]),
            "sgw": _round_f32r(sgT[:, hs]),
            "suw": _round_f32r(suT[:, hs]),
            "sdw": _round_f32r(sdT[hs, :]),
        })
    return in_maps


def kernel(x, router_w, correction_bias, gate_w, up_w, down_w,
           shared_gate_w, shared_up_w, shared_down_w):
    in_maps = make_in_maps(x, router_w, correction_bias, gate_w, up_w, down_w,
                           shared_gate_w, shared_up_w, shared_down_w)
    nc = _get_nc()
    res = run_bass_kernel_spmd(nc, in_maps, list(range(NCORES)))
    acc = np.zeros((C, S), np.float64)
    for c in range(NCORES):
        acc += res.results[c]["out"].astype(np.float64)
    return np.ascontiguousarray(acc.T).astype(np.float32).reshape(B, T, C)


# revision 21
# speedup vs baseline: 1.1175x; 1.0016x over previous
"""MoE FFN (grouped sigmoid top-k routing + shared expert) on 8 TRN2 NeuronCores.

Strategy: expert-parallel. Each core gets 2 of 16 routed experts plus 1/8 of
the shared expert (sharded along its hidden dim HS). x is replicated
(host-pre-transposed to [C, S] so every matmul contracts over the SBUF
partition dim). Routing is computed on-device, replicated on every core.
Each core emits a partial output [C, S]; the host sums the 8 partials and
transposes back.

dtypes: router matmuls run in full fp32 (top-k selection is sensitive to
input rounding); FFN matmuls run in fp32r (fp32 rounded to 11 mantissa bits,
full PE rate, ~1e-4 relative error).
"""

import numpy as np

import concourse.bacc as bacc
import concourse.mybir as mybir
from concourse import tile
from concourse.bass_utils import run_bass_kernel_spmd
from concourse.masks import make_identity

F32 = mybir.dt.float32
F32R = mybir.dt.float32r
AF = mybir.ActivationFunctionType
OP = mybir.AluOpType

# problem shapes (hardcoded; kernel.py must be self-contained)
B, T, C, H, HS = 2, 1024, 1024, 256, 2048
E, G, EPG = 16, 4, 4
TOPK = 4
NCORES = 8
S = B * T                  # 2048 tokens
EPC = E // NCORES          # 2 experts per core
HSL = HS // NCORES         # 256 shared-hidden rows per core
KC = C // 128              # 8 contraction chunks
NT = S // 128              # 16 token chunks
NSC = S // 512             # 4 moving (token) chunks of 512
NHC = H // 128             # 2 h chunks (same for HSL)
NCC = C // 128             # 8 output-row chunks


def _round_f32r(x: np.ndarray) -> np.ndarray:
    """Round fp32 to fp32r (RNE to 11 mantissa bits) — matches TRN2 PE."""
    u = np.ascontiguousarray(x, dtype=np.float32).view(np.uint32)
    u = u + 0x7FF + ((u >> 12) & 1)
    u = u & np.uint32(0xFFFFF000)
    return u.view(np.float32)


def build():
    nc = bacc.Bacc(
        "TRN2",
        target_bir_lowering=False,
        debug=False,
        enable_asserts=True,
        num_devices=NCORES,
    )
    # ---- DRAM I/O (per core) ----
    x_d = nc.declare_dram_parameter("xT", [C, S], F32, isOutput=False)
    rw_d = nc.declare_dram_parameter("rw", [128, 128], F32, isOutput=False)
    bias_d = nc.declare_dram_parameter("bias", [1, E], F32, isOutput=False)
    esel_d = nc.declare_dram_parameter("esel", [E, EPC * 128], F32R,
                                       isOutput=False)
    gw_d = nc.declare_dram_parameter("gw", [EPC, C, H], F32R, isOutput=False)
    uw_d = nc.declare_dram_parameter("uw", [EPC, C, H], F32R, isOutput=False)
    dw_d = nc.declare_dram_parameter("dw", [EPC, H, C], F32R, isOutput=False)
    sgw_d = nc.declare_dram_parameter("sgw", [C, HSL], F32R, isOutput=False)
    suw_d = nc.declare_dram_parameter("suw", [C, HSL], F32R, isOutput=False)
    sdw_d = nc.declare_dram_parameter("sdw", [HSL, C], F32R, isOutput=False)
    out_d = nc.declare_dram_parameter("out", [C, S], F32, isOutput=True)

    with tile.TileContext(nc) as tc:
        _emit(nc, tc, x_d, rw_d, bias_d, esel_d, gw_d, uw_d, dw_d,
              sgw_d, suw_d, sdw_d, out_d)
    nc.finalize()
    return nc


def _emit(nc, tc, x_d, rw_d, bias_d, esel_d, gw_d, uw_d, dw_d,
          sgw_d, suw_d, sdw_d, out_d):
    consts = tc.alloc_tile_pool(name="consts", bufs=1)
    ident = consts.tile([128, 128], F32)
    make_identity(nc, ident[:])
    rw = consts.tile([128, 128], F32)
    nc.sync.dma_start(rw[:], rw_d[:])
    bias_sb = consts.tile([1, E], F32)
    nc.sync.dma_start(bias_sb[:], bias_d[:])
    esel = consts.tile([E, EPC * 128], F32R)
    nc.sync.dma_start(esel[:], esel_d[:])
    # down-proj weights, resident (all 3 sources needed together in the
    # down pass): wd[src][p, hc*1024 + c] = dw[src][hc*128+p, c]
    wd = [consts.tile([128, NHC * C], F32R, tag=f"wd{i}", name=f"wd{i}")
          for i in range(3)]
    comb = consts.tile([128, NT * E], F32)       # combine weights [s, (t e)]

    # hw tiles [128, S] fp32r: (src, hc) -> silu(g)*u (* combine weight)
    hw_pool = tc.alloc_tile_pool(name="hw", bufs=1)
    hw = [[hw_pool.tile([128, S], F32R, tag=f"hw{src}{hc}",
                        name=f"hw{src}{hc}")
           for hc in range(NHC)] for src in range(3)]

    # x_r: fp32r copy of x, resident for all FFN matmuls
    xr_pool = tc.alloc_tile_pool(name="xr", bufs=1)
    x_r = xr_pool.tile([128, KC * S], F32R)

    # gate/up weight pool (opened early so expert 0's weights stream in
    # behind the first x chunk, during the router phase)
    wp = tc.alloc_tile_pool(name="wp", bufs=2)
    w_tiles = {}

    def load_w(src):
        # one [128, KC*128] tile per (proj, hc): finer slot rotation lets the
        # next source's first-half weights stream while the current source is
        # still computing its second half
        tiles = {}
        for proj, wsrc in (("g", gw_d[src] if src < 2 else sgw_d),
                           ("u", uw_d[src] if src < 2 else suw_d)):
            for hc in range(NHC):
                wt = wp.tile([128, KC * 128], F32R, tag=f"{proj}{hc}",
                             name=f"w{proj}{src}{hc}")
                nc.sync.dma_start(
                    wt.rearrange("p (k h) -> p k h", k=KC),
                    wsrc.rearrange("(k p) h -> p k h", p=128)[
                        :, :, hc * 128:(hc + 1) * 128])
                tiles[(proj, hc)] = wt
        w_tiles[src] = tiles

    # ---------------- router + routing (scoped pools) ----------------
    with (
        tc.tile_pool(name="rt", bufs=1) as rt,
        tc.tile_pool(name="xs", bufs=2) as xs,
        tc.tile_pool(name="psl", bufs=NSC, space="PSUM") as psl,
        tc.tile_pool(name="pst", bufs=2, space="PSUM") as pst,
    ):
        scoresT = rt.tile([E, S], F32)
        pl = [psl.tile([E, 512], F32, tag="pl", name=f"pl{i}") for i in range(NSC)]
        HS2 = S // 2
        for k in range(KC):
            # two half-chunk tiles with separate tags: the WAR on slot reuse
            # releases per half, so the DMA stream runs ahead of the PE
            xlo = xs.tile([128, HS2], F32, tag="xkl", name="xlo", bufs=3)
            xhi = xs.tile([128, HS2], F32, tag="xkh", name="xhi")
            eng = nc.sync if k % 2 == 0 else nc.gpsimd
            oth = nc.gpsimd if k % 2 == 0 else nc.sync
            if k == 0:
                nc.sync.dma_start(xlo[:, :512], x_d[:128, :512])
                nc.gpsimd.dma_start(xlo[:, 512:], x_d[:128, 512:HS2])
                nc.sync.dma_start(xhi[:], x_d[:128, HS2:])
            else:
                eng.dma_start(xlo[:], x_d[k * 128:(k + 1) * 128, :HS2])
                oth.dma_start(xhi[:], x_d[k * 128:(k + 1) * 128, HS2:])
            # fp32r rounding copies for the FFN path
            nc.vector.tensor_copy(x_r[:, k * S:k * S + HS2], xlo[:])
            nc.vector.tensor_copy(x_r[:, k * S + HS2:(k + 1) * S], xhi[:])
            for sc in range(NSC):
                src_t = xlo if sc < 2 else xhi
                nc.tensor.matmul(
                    pl[sc][:],
                    rw[:, k * E:(k + 1) * E],
                    src_t[:, (sc % 2) * 512:(sc % 2 + 1) * 512],
                    start=(k == 0), stop=(k == KC - 1),
                )
        load_w(0)
        for sc in range(NSC):
            nc.scalar.activation(scoresT[:, sc * 512:(sc + 1) * 512], pl[sc][:],
                                 AF.Sigmoid)

        # transpose scores -> [s, (t e)] layout
        scores = rt.tile([128, NT * E], F32)
        for t in range(NT):
            pt = pst.tile([128, E], F32, tag="pt")
            nc.tensor.transpose(pt[:], scoresT[:, t * 128:(t + 1) * 128],
                                ident[:E, :E])
            nc.vector.tensor_copy(scores[:, t * E:(t + 1) * E], pt[:])

        # ---- routing math (all DVE), layout [128, (t=16, e=16)] ----
        sb = rt.tile([128, NT * E], F32)
        bias_exp = rt.tile([128, E], F32)
        nc.gpsimd.partition_broadcast(bias_exp[:], bias_sb[0:1, :])
        sbv = sb.rearrange("p (t e) -> p t e", t=NT)
        scv = scores.rearrange("p (t e) -> p t e", t=NT)
        nc.vector.tensor_add(
            sbv, scv, bias_exp[:, None, :].to_broadcast([128, NT, E]))

        # group top-2 sum over each group of 4: max over the 6 pairwise sums
        sbg = sb.rearrange("p (t g j) -> p t g j", t=NT, g=G)
        t2s = rt.tile([128, NT * G], F32)
        t2sv = t2s.rearrange("p (t g) -> p t g", t=NT)
        tmp = rt.tile([128, NT * G], F32)
        tmpv = tmp.rearrange("p (t g) -> p t g", t=NT)
        pairs = [(a, b) for a in range(EPG) for b in range(a + 1, EPG)]
        first = True
        for (a, b) in pairs:
            dst = t2sv if first else tmpv
            nc.vector.tensor_add(dst, sbg[:, :, :, a], sbg[:, :, :, b])
            if not first:
                nc.vector.tensor_max(t2sv, t2sv, tmpv)
            first = False

        # second-largest group score per token: max over pairwise mins
        m2 = rt.tile([128, NT], F32)
        m2t = rt.tile([128, NT], F32)
        gpairs = [(a, b) for a in range(G) for b in range(a + 1, G)]
        first = True
        for (a, b) in gpairs:
            dst = m2 if first else m2t
            nc.vector.tensor_tensor(dst[:], t2sv[:, :, a], t2sv[:, :, b], OP.min)
            if not first:
                nc.vector.tensor_max(m2[:], m2[:], m2t[:])
            first = False

        # penalty: -1e30 on experts whose group is not in the top 2
        pen = rt.tile([128, NT * G], F32)
        penv = pen.rearrange("p (t g) -> p t g", t=NT)
        nc.vector.tensor_tensor(
            penv, t2sv, m2[:, :, None].to_broadcast([128, NT, G]), OP.is_lt)
        nc.vector.tensor_scalar_mul(pen[:], pen[:], -1e30)

        sbm = rt.tile([128, NT * E], F32)
        sbmg = sbm.rearrange("p (t g j) -> p t g j", t=NT, g=G)
        nc.vector.tensor_add(
            sbmg, sbg, penv[:, :, :, None].to_broadcast([128, NT, G, EPG]))

        # 4th largest of the masked biased scores per token -> threshold
        m8 = rt.tile([128, NT * 8], F32)
        for t in range(NT):
            nc.vector.max(m8[:, t * 8:(t + 1) * 8], sbm[:, t * E:(t + 1) * E])
        v4 = m8.rearrange("p (t k) -> p t k", t=NT)[:, :, TOPK - 1]

        msk = rt.tile([128, NT * E], F32)
        mskv = msk.rearrange("p (t e) -> p t e", t=NT)
        sbmv = sbm.rearrange("p (t e) -> p t e", t=NT)
        nc.vector.tensor_tensor(
            mskv, sbmv, v4[:, :, None].to_broadcast([128, NT, E]), OP.is_ge)

        # weights: unbiased scores at selected positions, renormalized
        wm = rt.tile([128, NT * E], F32)
        nc.vector.tensor_mul(wm[:], scores[:], msk[:])
        ws = rt.tile([128, NT], F32)
        nc.vector.reduce_sum(ws[:], wm.rearrange("p (t e) -> p t e", t=NT),
                             axis=mybir.AxisListType.X)
        nc.vector.tensor_scalar_add(ws[:], ws[:], 1e-20)
        wr = rt.tile([128, NT], F32)
        nc.vector.reciprocal(wr[:], ws[:])
        combv = comb.rearrange("p (t e) -> p t e", t=NT)
        nc.vector.tensor_mul(
            combv, wm.rearrange("p (t e) -> p t e", t=NT),
            wr[:, :, None].to_broadcast([128, NT, E]))

    # ---------------- FFN ----------------
    # down-proj weight loads (needed only in the down pass; emitted here so
    # they don't delay the x/router DMAs)
    for src in range(2):
        nc.sync.dma_start(
            wd[src].rearrange("p (hc c) -> p hc c", hc=NHC),
            dw_d[src].rearrange("(hc p) c -> p hc c", p=128))
    nc.sync.dma_start(
        wd[2].rearrange("p (hc c) -> p hc c", hc=NHC),
        sdw_d.rearrange("(hc p) c -> p hc c", p=128))

    cp = tc.alloc_tile_pool(name="cp", bufs=1)
    with (
        tc.tile_pool(name="cb", bufs=1) as cbp,
        tc.tile_pool(name="hsb", bufs=2) as hsb,
        tc.tile_pool(name="psg", bufs=3, space="PSUM") as psg,
        tc.tile_pool(name="psu", bufs=3, space="PSUM") as psu,
    ):
        combT = None
        for src in range(3):
            if src not in w_tiles:
                load_w(src)
            wt = w_tiles.pop(src)

            for hc in range(NHC):
                h_sb = hsb.tile([128, S], F32, tag="h")
                for sc in range(NSC):
                    pg = psg.tile([128, 512], F32, tag="pg")
                    pu = psu.tile([128, 512], F32, tag="pu")
                    for k in range(KC):
                        nc.tensor.matmul(
                            pg[:],
                            wt[("g", hc)][:, k * 128:(k + 1) * 128],
                            x_r[:, k * S + sc * 512: k * S + (sc + 1) * 512],
                            start=(k == 0), stop=(k == KC - 1))
                    for k in range(KC):
                        nc.tensor.matmul(
                            pu[:],
                            wt[("u", hc)][:, k * 128:(k + 1) * 128],
                            x_r[:, k * S + sc * 512: k * S + (sc + 1) * 512],
                            start=(k == 0), stop=(k == KC - 1))
                    sl = slice(sc * 512, (sc + 1) * 512)
                    nc.scalar.activation(h_sb[:, sl], pg[:], AF.Silu)
                    if src == 2:
                        # shared expert: no combine scaling; write f32r directly
                        nc.vector.tensor_mul(hw[src][hc][:, sl], h_sb[:, sl],
                                             pu[:])
                    else:
                        nc.vector.tensor_mul(h_sb[:, sl], h_sb[:, sl], pu[:])

                if src == 0 and combT is None:
                    # emit combine transposes after the first expert's g/u
                    # matmuls so the PE isn't stalled on the routing DVE chain
                    combT = cp.tile([E, S], F32R)
                    with tc.tile_pool(name="psct", bufs=2,
                                      space="PSUM") as psc:
                        for t in range(NT):
                            pct = psc.tile([E, 128], F32, tag="pct")
                            nc.tensor.transpose(
                                pct[:], comb[:, t * E:(t + 1) * E], ident[:])
                            nc.vector.tensor_copy(
                                combT[:, t * 128:(t + 1) * 128], pct[:])

                if src < 2 and hc == 0:
                    # broadcast this core's combine row across partitions by
                    # multiplying with a column-replicated one-hot (PE)
                    cb_exp = cbp.tile([128, S], F32, tag="cb", name="cb_exp")
                    with tc.tile_pool(name="pse2", bufs=2,
                                      space="PSUM") as pse2p:
                        for sc in range(NSC):
                            pe2 = pse2p.tile([128, 512], F32, tag="pe2")
                            nc.tensor.matmul(
                                pe2[:], esel[:, src * 128:(src + 1) * 128],
                                combT[:, sc * 512:(sc + 1) * 512],
                                start=True, stop=True)
                            nc.vector.tensor_copy(
                                cb_exp[:, sc * 512:(sc + 1) * 512], pe2[:])
                    cb_cur = cb_exp

                if src < 2:
                    nc.vector.tensor_mul(hw[src][hc][:], h_sb[:], cb_cur[:])

    cp.release()
    wp.release()
    xr_pool.release()

    # ---------------- down projection ----------------
    with (
        tc.tile_pool(name="oso", bufs=2) as oso,
        tc.tile_pool(name="pso", bufs=4, space="PSUM") as pso,
    ):
        for cc in range(NCC):
            os_t = oso.tile([128, S], F32, tag="os")
            for sc in range(NSC):
                po = pso.tile([128, 512], F32, tag="po")
                idx = 0
                for src in range(3):
                    for hc in range(NHC):
                        nc.tensor.matmul(
                            po[:],
                            wd[src][:, hc * C + cc * 128: hc * C + (cc + 1) * 128],
                            hw[src][hc][:, sc * 512:(sc + 1) * 512],
                            start=(idx == 0), stop=(idx == 5))
                        idx += 1
                nc.vector.tensor_copy(os_t[:, sc * 512:(sc + 1) * 512], po[:])
                if cc == NCC - 1:
                    oeng = nc.sync if sc % 2 == 0 else nc.gpsimd
                    oeng.dma_start(
                        out_d[cc * 128:(cc + 1) * 128,
                              sc * 512:(sc + 1) * 512],
                        os_t[:, sc * 512:(sc + 1) * 512])
            if cc < NCC - 1:
                nc.sync.dma_start(out_d[cc * 128:(cc + 1) * 128, :], os_t[:])

    hw_pool.release()
    consts.release()


_NC_CACHE = {}


def _get_nc():
    if "nc" not in _NC_CACHE:
        _NC_CACHE["nc"] = build()
    return _NC_CACHE["nc"]


def make_in_maps(x, router_w, correction_bias, gate_w, up_w, down_w,
                 shared_gate_w, shared_up_w, shared_down_w):
    x = np.asarray(x, dtype=np.float32)
    xT = np.ascontiguousarray(x.reshape(S, C).T)                 # [C, S]
    rwT = np.asarray(router_w, dtype=np.float32).T               # [C, E]
    rw_pk = np.ascontiguousarray(
        rwT.reshape(KC, 128, E).transpose(1, 0, 2).reshape(128, KC * E))
    bias = np.asarray(correction_bias, dtype=np.float32).reshape(1, E)
    sgT = np.asarray(shared_gate_w, dtype=np.float32).T          # [C, HS]
    suT = np.asarray(shared_up_w, dtype=np.float32).T            # [C, HS]
    sdT = np.asarray(shared_down_w, dtype=np.float32).T          # [HS, C]
    gate_w = np.asarray(gate_w, dtype=np.float32)
    up_w = np.asarray(up_w, dtype=np.float32)
    down_w = np.asarray(down_w, dtype=np.float32)

    in_maps = []
    for c in range(NCORES):
        es = slice(c * EPC, (c + 1) * EPC)
        hs = slice(c * HSL, (c + 1) * HSL)
        esel = np.zeros((E, EPC * 128), np.float32)
        esel[c * EPC, 0:128] = 1.0
        esel[c * EPC + 1, 128:256] = 1.0
        in_maps.append({
            "xT": xT,
            "rw": rw_pk,
            "bias": bias,
            "esel": esel,
            "gw": _round_f32r(gate_w[es]),
            "uw": _round_f32r(up_w[es]),
            "dw": _round_f32r(down_w[es]),
            "sgw": _round_f32r(sgT[:, hs]),
            "suw": _round_f32r(suT[:, hs]),
            "sdw": _round_f32r(sdT[hs, :]),
        })
    return in_maps


def kernel(x, router_w, correction_bias, gate_w, up_w, down_w,
           shared_gate_w, shared_up_w, shared_down_w):
    in_maps = make_in_maps(x, router_w, correction_bias, gate_w, up_w, down_w,
                           shared_gate_w, shared_up_w, shared_down_w)
    nc = _get_nc()
    res = run_bass_kernel_spmd(nc, in_maps, list(range(NCORES)))
    acc = np.zeros((C, S), np.float64)
    for c in range(NCORES):
        acc += res.results[c]["out"].astype(np.float64)
    return np.ascontiguousarray(acc.T).astype(np.float32).reshape(B, T, C)
